# revision 12
# baseline (speedup 1.0000x reference)
"""GATNet (3-layer GAT with edge features) on 8 Trainium2 NeuronCores.

Strategy (dst-sharded, edge-sorted, host-assisted):
  - Nodes padded to N_PAD=20480 -> 160 chunks of 128 nodes; 20 chunks/core.
  - Edges + one self-loop per node, sorted by dst; every edge lands on the
    core owning its dst chunk => all segment softmax/aggregation core-local.
  - Host precomputes (not on the device critical path):
      * a_e = edge_attr @ (We folded with att_e) for all 3 layers, incl. the
        self-loop rows (PyG fill_value='mean' -> per-dst mean of real a_e)
        and -1e4 on padding slots (kills exp); staged as bf16 input.
      * one-hot scatter matrices M1 (edge->dst, fp8, SBUF-resident) and M1T
        (dst->edge, fp8, streamed per chunk) for the PE-matmul scatter/
        broadcast; identity I128 fp8.
      * layer-1 projections: hs1 table [h|a_s] (bf16, replicated) and a_d1.
  - Per layer: (L2/L3 only) cores project their own 2560-node shard with one
    fused matmul (att_s/att_d folded into W on host), AllGather the bf16
    [h|a_s] table; then per chunk:
      dma_gather h-rows by src; alpha = M1T@a_d + I@a_e + I@a_s accumulated
      on the PE into PSUM; DVE leaky-relu; Act engine exp with C-broadcast
      (L3 split Act/Pool); DVE 2x multiply exh = ex*h in place; PE matmuls
      M1^T @ [exh|ex] scatter-add numerator+denominator into PSUM; DVE
      divide+relu.  Softmax max-subtraction dropped (alpha is O(1)).
"""

import math
import sys

import numpy as np

sys.path.insert(0, "/opt/trn_rl_repo")

import ml_dtypes  # noqa: E402

import concourse.bacc as bacc  # noqa: E402
import concourse.bass as bass  # noqa: E402
import concourse.mybir as mybir  # noqa: E402
import concourse.tile as tile  # noqa: E402
from concourse.bass_utils import run_bass_kernel_spmd  # noqa: E402

bf16 = ml_dtypes.bfloat16
fp8 = ml_dtypes.float8_e3m4

N = 20000
E = 320000
FIN = 16
ED = 22
NEG = 0.2
NCORES = 8
NPAD = 20480
NCH = NPAD // 128 // NCORES     # 20 chunks per core
SHARD = NPAD // NCORES          # 2560 own nodes per core
LAYERS = [(16, 8, 32), (256, 8, 32), (256, 12, 64)]
ROWW = [384, 384, 896]          # hs row: [h(HC) | a_s(H) | pad], bytes%256==0
AEW = [8, 8, 12]
AE_TOT = 28
F32 = mybir.dt.float32
BF16 = mybir.dt.bfloat16
FP8 = mybir.dt.float8e3
I16 = mybir.dt.int16
AF = mybir.ActivationFunctionType
ALU = mybir.AluOpType
L3_ACT_FRAC = 0.62              # fraction of L3 ex-broadcast done on Act


# ============================ host-side prep ============================

def _prep_graph(edge_index):
    src = edge_index[0].astype(np.int64)
    dst = edge_index[1].astype(np.int64)
    order = np.argsort(dst, kind="stable")
    src_s = src[order]
    dst_s = dst[order]
    cnt = np.bincount(dst, minlength=NPAD)

    nchunks = NPAD // 128
    chunk_of = dst_s // 128
    chunk_cnt = np.bincount(chunk_of, minlength=nchunks) + 128
    NT = int(math.ceil(chunk_cnt.max() / 128.0))
    SL = NT * 128

    tot = nchunks * SL
    g_src = np.zeros(tot, np.int64)
    g_dstloc = np.zeros(tot, np.int64)
    g_isself = np.zeros(tot, np.bool_)
    g_eaidx = np.full(tot, -1, np.int64)

    starts = np.searchsorted(chunk_of, np.arange(nchunks))
    ends = np.searchsorted(chunk_of, np.arange(nchunks) + 1)
    for c in range(nchunks):
        base = c * SL
        r0, r1 = int(starts[c]), int(ends[c])
        d_loc = dst_s[r0:r1] - c * 128
        nreal = r1 - r0
        seg_cnt = np.bincount(d_loc, minlength=128)
        blk_off = np.zeros(128, np.int64)
        np.cumsum(seg_cnt[:-1] + 1, out=blk_off[1:])
        within = np.arange(nreal) - np.repeat(np.cumsum(seg_cnt) - seg_cnt, seg_cnt)
        pos = base + blk_off[d_loc] + within
        g_src[pos] = src_s[r0:r1]
        g_dstloc[pos] = d_loc
        g_eaidx[pos] = order[r0:r1]
        pos_self = base + blk_off + seg_cnt
        g_src[pos_self] = c * 128 + np.arange(128)
        g_dstloc[pos_self] = np.arange(128)
        g_isself[pos_self] = True

    g_ispad = (g_eaidx < 0) & ~g_isself
    return {"NT": NT, "SL": SL, "cnt": cnt, "src": g_src,
            "dstloc": g_dstloc, "isself": g_isself, "eaidx": g_eaidx,
            "ispad": g_ispad}


def _wrap_idx(idx):
    n = idx.shape[0]
    w = idx.astype(np.int16).reshape(n // 16, 16).T
    return np.tile(w, (8, 1))


def _prep_params(kw):
    p = {}
    wered = []
    for li, (fin, H, C) in enumerate(LAYERS):
        i = li + 1
        W = kw[f"W{i}"].astype(np.float32)
        We = kw[f"We{i}"].astype(np.float32)
        ats = kw[f"as{i}"].astype(np.float32)
        atd = kw[f"ad{i}"].astype(np.float32)
        ate = kw[f"ae{i}"].astype(np.float32)
        Was = np.einsum("dhc,hc->dh", W.reshape(fin, H, C), ats)
        Wad = np.einsum("dhc,hc->dh", W.reshape(fin, H, C), atd)
        if li > 0:
            p[f"Wall{i}"] = np.concatenate([W, Was, Wad], axis=1).astype(bf16)
            bias = np.zeros((1, H * C + H), np.float32)
            bias[0, :H * C] = kw[f"b{i}"].astype(np.float32)
            p[f"bias{i}"] = bias.astype(bf16)
        else:
            # layer 1 projections are host-computed from x directly
            p["bias1"] = np.concatenate(
                [kw["b1"].astype(np.float32), np.zeros(H, np.float32)]
            ).reshape(1, -1).astype(bf16)
            p["_W1"] = W
            p["_Was1"] = Was
            p["_Wad1"] = Wad
        wered.append(np.einsum("dhc,hc->dh", We.reshape(ED, H, C), ate))
    p["_wered"] = np.concatenate(wered, axis=1).astype(np.float32)  # [ED,28]
    Wf = kw["Wf"].astype(np.float32).reshape(-1)
    p["wf1"] = np.ascontiguousarray(Wf[0:256].reshape(2, 128).T).astype(bf16)
    p["wf2"] = np.ascontiguousarray(Wf[256:512].reshape(2, 128).T).astype(bf16)
    p["wf3"] = np.ascontiguousarray(Wf[512:1280].reshape(6, 128).T).astype(bf16)
    p["bf"] = kw["bf"].astype(np.float32).reshape(1, 1)
    p["ident"] = np.eye(128, dtype=fp8)
    return p


def _prep_core_inputs(meta, x, edge_attr, params):
    NT, SL = meta["NT"], meta["SL"]
    npc = NCH * SL

    # ---- layer-1 host projections ----
    x_pad = np.zeros((NPAD, FIN), np.float32)
    x_pad[:N] = x
    h1 = x_pad @ params["_W1"]                        # [NPAD, 256]
    as1 = x_pad @ params["_Was1"]                     # [NPAD, 8]
    ad1 = x_pad @ params["_Wad1"]                     # [NPAD, 8]
    hs1 = np.zeros((NPAD, ROWW[0]), np.float32)
    hs1[:, 0:256] = h1
    hs1[:, 256:264] = as1
    hs1_b = hs1.astype(bf16)

    # ---- a_e for all edge slots (3 layers fused), incl. self means/pad ----
    ae_edge = edge_attr.astype(np.float32) @ params["_wered"]   # [E, 28]
    cnt = np.maximum(meta["cnt"][:, None], 1.0)
    sums = np.zeros((NPAD, AE_TOT), np.float32)
    dst_full = np.zeros(0)
    # mean per dst over real incoming edges
    # (use eaidx/dstloc info: real slots have eaidx>=0)
    real = meta["eaidx"] >= 0
    slot_chunk = np.arange(160 * SL) // SL
    dst_node = slot_chunk * 128 + meta["dstloc"]
    np.add.at(sums, dst_node[real], ae_edge[meta["eaidx"][real]])
    mean_ae = sums / cnt
    ae_slot = np.zeros((160 * SL, AE_TOT), np.float32)
    ae_slot[real] = ae_edge[meta["eaidx"][real]]
    ae_slot[meta["isself"]] = mean_ae[dst_node[meta["isself"]]]
    ae_slot[meta["ispad"]] = -1e4
    # layer-1 a_d[dst] is host-known: fold it into layer-1 a_e
    ae_slot[:, 0:8] += ad1[dst_node]

    # ---- one-hot scatter matrices ----
    dl = meta["dstloc"].reshape(160, NT, 128)         # [chunk, t, e]
    eye = np.eye(128, dtype=fp8)
    m1_all = eye[dl]                                  # [chunk, t, e(part), n]
    ins = []
    for r in range(NCORES):
        sl = slice(r * npc, (r + 1) * npc)
        chs = slice(r * NCH, (r + 1) * NCH)
        idx16 = _wrap_idx(meta["src"][sl])
        # M1: [128(e), NCH*NT*128(n)]
        m1 = np.ascontiguousarray(
            m1_all[chs].transpose(2, 0, 1, 3).reshape(128, NCH * NT * 128))
        # M1T: [128(n), NCH*NT*128(e)]
        m1t = np.ascontiguousarray(
            m1_all[chs].transpose(3, 0, 1, 2).reshape(128, NCH * NT * 128))
        d = {"idx16": idx16, "m1": m1, "m1t": m1t, "hs1": hs1_b}
        off = 0
        for li, w in enumerate(AEW):
            ae_c = np.ascontiguousarray(
                ae_slot[sl, off:off + w].reshape(NCH * NT, 128, w)
                .transpose(1, 0, 2).reshape(128, NCH * NT * w)).astype(bf16)
            d[f"ae{li + 1}c"] = ae_c
            off += w
        d.update({k: v for k, v in params.items() if not k.startswith("_")})
        ins.append(d)
    return ins


# ============================ device kernel ============================

def build_kernel(NT, nch=NCH, use_cc=True):
    NCHl = nch
    SHARDl = NCHl * 128
    NPADl = SHARDl * NCORES
    SL = NT * 128
    npc = NCHl * SL
    TPC = NCHl * NT

    nc = bacc.Bacc("TRN2", num_devices=NCORES)

    d_idx = nc.dram_tensor("idx16", [128, npc // 16], I16, kind="ExternalInput")
    d_m1 = nc.dram_tensor("m1", [128, npc], FP8, kind="ExternalInput")
    d_m1t = nc.dram_tensor("m1t", [128, npc], FP8, kind="ExternalInput")
    d_aec = [nc.dram_tensor(f"ae{li + 1}c", [128, TPC * AEW[li]], BF16,
                            kind="ExternalInput") for li in range(3)]
    d_hs1 = nc.dram_tensor("hs1", [NPADl, ROWW[0]], BF16, kind="ExternalInput")
    d_ident = nc.dram_tensor("ident", [128, 128], FP8, kind="ExternalInput")
    d_Wall, d_bias = {}, {}
    for li, (fin, H, C) in enumerate(LAYERS):
        if li > 0:
            d_Wall[li] = nc.dram_tensor(f"Wall{li + 1}", [fin, H * C + 2 * H],
                                        BF16, kind="ExternalInput")
        d_bias[li] = nc.dram_tensor(f"bias{li + 1}", [1, H * C + H], BF16,
                                    kind="ExternalInput")
    d_wf = [nc.dram_tensor(f"wf{i + 1}", [128, nb], BF16, kind="ExternalInput")
            for i, nb in enumerate((2, 2, 6))]
    d_bf = nc.dram_tensor("bf", [1, 1], F32, kind="ExternalInput")
    d_y = nc.dram_tensor("y", [1, SHARDl], F32, kind="ExternalOutput")

    with tile.TileContext(nc) as tc:
        with tc.tile_pool(name="const", bufs=1) as cpool, \
             tc.tile_pool(name="lay", bufs=1) as lpool, \
             tc.tile_pool(name="work", bufs=2) as wpool, \
             tc.tile_pool(name="gbuf", bufs=2) as gpool, \
             tc.tile_pool(name="exbuf", bufs=2) as xpool, \
             tc.tile_pool(name="psbig", bufs=2, space="PSUM") as psb, \
             tc.tile_pool(name="pssm", bufs=2, space="PSUM") as pss:

            # internal DRAM
            d_hs_in = [None] + [nc.dram_tensor(f"d_hs_in{li}", [SHARDl, ROWW[li]],
                                               BF16) for li in (1, 2)]
            d_hs = [None] + [nc.dram_tensor(f"d_hs{li}", [NPADl, ROWW[li]], BF16)
                             for li in (1, 2)]
            d_x = [nc.dram_tensor(f"d_x{li}",
                                  [SHARDl, LAYERS[li][1] * LAYERS[li][2]], BF16)
                   for li in range(3)]

            # ---------- constants ----------
            t_ones = cpool.tile([1, 128], BF16)
            nc.vector.memset(t_ones[:], 1.0)
            t_ident = cpool.tile([128, 128], FP8)
            nc.sync.dma_start(out=t_ident[:], in_=d_ident[:])
            t_idx = cpool.tile([128, npc // 16], I16)
            nc.sync.dma_start(out=t_idx[:], in_=d_idx[:])
            t_m1 = cpool.tile([128, npc], FP8)
            nc.sync.dma_start(out=t_m1[:], in_=d_m1[:])

            # ---------- layers ----------
            for li, (fin, H, C) in enumerate(LAYERS):
                HC = H * C
                RW_ = ROWW[li]
                AEw = AE_TOT
                NDW = HC + H
                PJW = HC + 2 * H
                nkb = max(fin // 128, 1)
                KP = min(fin, 128)

                t_bias = lpool.tile([1, NDW], BF16, tag=f"bias{li}")
                nc.sync.dma_start(out=t_bias[:], in_=d_bias[li][:])
                t_ad = lpool.tile([128, NCHl * H], BF16, tag=f"ad{li}")

                # ---- phase A: own-shard projections -> hs shard + a_d ----
                if li == 0:
                    hs_tab = d_hs1
                else:
                    t_W = lpool.tile([KP, nkb, PJW], BF16, tag=f"W{li}")
                    nc.sync.dma_start(
                        out=t_W[:],
                        in_=d_Wall[li][:].rearrange("(b p) w -> p b w", p=KP))
                    for ch in range(NCHl):
                        t_xc = wpool.tile([128, nkb * 128], BF16, tag="xc")
                        for b in range(nkb):
                            nc.sync.dma_start(
                                out=t_xc[:, b * 128:(b + 1) * 128],
                                in_=d_x[li - 1][ch * 128:(ch + 1) * 128,
                                                b * 128:(b + 1) * 128],
                                transpose=True)
                        p_h = psb.tile([128, 1024], F32, space="PSUM", tag="big")
                        for b in range(nkb):
                            xsl = t_xc[:KP, b * 128:(b + 1) * 128]
                            for c0 in range(0, PJW, 512):
                                c1 = min(c0 + 512, PJW)
                                nc.tensor.matmul(
                                    out=p_h[:, c0:c1],
                                    lhsT=xsl,
                                    rhs=t_W[:, b, c0:c1],
                                    start=(b == 0), stop=(b == nkb - 1))
                        t_hs = wpool.tile([128, RW_], BF16, tag="hsrow")
                        nc.scalar.copy(out=t_hs[:, 0:HC + H], in_=p_h[:, 0:HC + H])
                        nc.sync.dma_start(out=d_hs_in[li][ch * 128:(ch + 1) * 128, :],
                                          in_=t_hs[:])
                        nc.vector.tensor_copy(
                            t_ad[:, ch * H:(ch + 1) * H],
                            p_h[:, HC + H:HC + 2 * H])

                    if use_cc:
                        nc.gpsimd.collective_compute(
                            "AllGather", ALU.bypass,
                            replica_groups=[list(range(NCORES))],
                            ins=[d_hs_in[li].ap().opt()],
                            outs=[d_hs[li].ap().opt()])
                    else:
                        nc.gpsimd.dma_start(out=d_hs[li][0:SHARDl, :],
                                            in_=d_hs_in[li][:])
                    hs_tab = d_hs[li]

                # ---- phase B: edges ----
                for ch in range(NCHl):
                    t_g = gpool.tile([128, NT, RW_], BF16, tag="G")
                    nc.gpsimd.dma_gather(t_g[:], hs_tab[:],
                                         t_idx[:, ch * SL // 16:(ch + 1) * SL // 16],
                                         SL, SL, RW_, single_packet=False)
                    AEw_l = AEW[li]
                    t_ae = wpool.tile([128, NT, AEw_l], BF16, tag="aeL")
                    nc.sync.dma_start(
                        out=t_ae[:],
                        in_=d_aec[li][:, ch * NT * AEw_l:(ch + 1) * NT * AEw_l]
                        .rearrange("p (t h) -> p t h", t=NT))
                    if li > 0:
                        t_m1t = wpool.tile([128, SL], FP8, tag="m1t")
                        nc.sync.dma_start(out=t_m1t[:],
                                          in_=d_m1t[:, ch * SL:(ch + 1) * SL])

                    # three passes over half-chunks so each engine's in-order
                    # queue never parks behind a cross-engine wait:
                    #   pass 1 (PE): alpha = a_d[dst] + a_e + a_s[src]
                    #   pass 2 (DVE/Act/Pool): lrelu -> exp + C-replicate
                    #   pass 3 (DVE then PE): exh multiply -> scatter matmuls
                    NH = (NT + 1) // 2
                    halves = [(0, NH), (NH, NT)]
                    p_als, t_lrs, t_exCs = [], [], []
                    for h0, h1 in halves:
                        nh = h1 - h0
                        p_al = pss.tile([128, NH, H], F32, space="PSUM",
                                        tag="psmA")
                        p_als.append(p_al)
                        for t in range(h0, h1):
                            tt = t - h0
                            if li > 0:
                                nc.tensor.matmul(
                                    out=p_al[:, tt, :],
                                    lhsT=t_m1t[:, t * 128:(t + 1) * 128],
                                    rhs=t_ad[:, ch * H:(ch + 1) * H],
                                    start=True, stop=False)
                            nc.tensor.matmul(out=p_al[:, tt, :],
                                             lhsT=t_ident[:],
                                             rhs=t_ae[:, t, :],
                                             start=(li == 0), stop=False)
                            nc.tensor.matmul(out=p_al[:, tt, :],
                                             lhsT=t_ident[:],
                                             rhs=t_g[:, t, HC:HC + H],
                                             start=False, stop=True)
                    for hi, (h0, h1) in enumerate(halves):
                        nh = h1 - h0
                        p_al = p_als[hi]
                        t_al = wpool.tile([128, NH, H], F32, tag="al")
                        nc.vector.tensor_copy(t_al[:, 0:nh], p_al[:, 0:nh])
                        t_lr = wpool.tile([128, NH, H], F32, tag="lr")
                        nc.vector.scalar_tensor_tensor(
                            out=t_lr[:, 0:nh], in0=t_al[:, 0:nh], scalar=NEG,
                            in1=t_al[:, 0:nh], op0=ALU.mult, op1=ALU.max)
                        t_exC = xpool.tile([128, NH, H, C], BF16, tag="exC")
                        t_exCs.append(t_exC)
                        if li < 2:
                            nc.scalar.activation(
                                t_exC[:, 0:nh],
                                t_lr[:, 0:nh].unsqueeze(-1)
                                .broadcast_to([128, nh, H, C]),
                                AF.Exp)
                        else:
                            t_ex = wpool.tile([128, NH, H], BF16, tag="ex")
                            nc.scalar.activation(t_ex[:, 0:nh], t_lr[:, 0:nh],
                                                 AF.Exp)
                            ta = max(1, int(nh * L3_ACT_FRAC))
                            nc.scalar.copy(
                                out=t_exC[:, 0:ta],
                                in_=t_ex[:, 0:ta].unsqueeze(-1)
                                .broadcast_to([128, ta, H, C]))
                            nc.gpsimd.tensor_copy(
                                t_exC[:, ta:nh],
                                t_ex[:, ta:nh].unsqueeze(-1)
                                .broadcast_to([128, nh - ta, H, C]))

                    p_nd = psb.tile([128, 1024], F32, space="PSUM", tag="big")
                    for c0 in range(0, NDW, 512):
                        c1 = min(c0 + 512, NDW)
                        nc.tensor.matmul(out=p_nd[:, c0:c1], lhsT=t_ones[:],
                                         rhs=t_bias[:, c0:c1], start=True,
                                         stop=False)
                    m1base = ch * SL
                    for hi, (h0, h1) in enumerate(halves):
                        nh = h1 - h0
                        t_exC = t_exCs[hi]
                        # exh = ex*h in place (2x mode: all bf16 packed)
                        nc.vector.tensor_tensor(
                            out=t_g[:, h0:h1, 0:HC],
                            in0=t_g[:, h0:h1, 0:HC],
                            in1=t_exC[:, 0:nh].rearrange("p t h c -> p t (h c)"),
                            op=ALU.mult)
                        # denominator cols: ex (pick c=0 stride-C view)
                        nc.vector.tensor_copy(t_g[:, h0:h1, HC:NDW],
                                              t_exC[:, 0:nh, :, 0])
                        # scatter this half: NUMDEN += sum_t M1_t.T @ exh_t
                        for t in range(h0, h1):
                            for c0 in range(0, NDW, 512):
                                c1 = min(c0 + 512, NDW)
                                nc.tensor.matmul(
                                    out=p_nd[:, c0:c1],
                                    lhsT=t_m1[:, m1base + t * 128:
                                              m1base + (t + 1) * 128],
                                    rhs=t_g[:, t, c0:c1],
                                    start=False, stop=(t == NT - 1))
                    # x = relu(num/den)
                    t_rec = wpool.tile([128, H], F32, tag="rec")
                    nc.vector.reciprocal(t_rec[:], p_nd[:, HC:NDW])
                    t_x = wpool.tile([128, HC], BF16, tag="xout")
                    nc.vector.scalar_tensor_tensor(
                        out=t_x[:].rearrange("p (h c) -> p h c", h=H),
                        in0=p_nd[:, 0:HC].rearrange("p (h c) -> p h c", h=H),
                        scalar=0.0, op0=ALU.max, op1=ALU.mult,
                        in1=t_rec[:].unsqueeze(-1).broadcast_to([128, H, C]))
                    nc.sync.dma_start(out=d_x[li][ch * 128:(ch + 1) * 128, :],
                                      in_=t_x[:])

            # ---------- final: y = sigmoid(concat(x1,x2,x3) @ Wf + bf) ----------
            t_wf = [lpool.tile([128, nb], BF16, tag=f"wf{i}", name=f"t_wf{i}")
                    for i, nb in enumerate((2, 2, 6))]
            for i in range(3):
                nc.sync.dma_start(out=t_wf[i][:], in_=d_wf[i][:])
            t_bf = lpool.tile([1, 1], F32, tag="bf")
            nc.sync.dma_start(out=t_bf[:], in_=d_bf[:])
            for g in range(SHARDl // 512):
                p_y = pss.tile([1, 512], F32, space="PSUM", tag="psmB")
                first = True
                for li in range(3):
                    nbl = (LAYERS[li][1] * LAYERS[li][2]) // 128
                    for b in range(nbl):
                        t_xg = wpool.tile([128, 512], BF16, tag="xg")
                        nc.sync.dma_start(
                            out=t_xg[:],
                            in_=d_x[li][g * 512:(g + 1) * 512,
                                        b * 128:(b + 1) * 128],
                            transpose=True)
                        nc.tensor.matmul(out=p_y[:], lhsT=t_wf[li][:, b:b + 1],
                                         rhs=t_xg[:], start=first,
                                         stop=(li == 2 and b == nbl - 1))
                        first = False
                t_y = wpool.tile([1, 512], F32, tag="yrow")
                nc.scalar.activation(t_y[:], p_y[:], AF.Sigmoid, bias=t_bf[:])
                nc.sync.dma_start(out=d_y[0:1, g * 512:(g + 1) * 512], in_=t_y[:])

    return nc


# ============================ public entry ============================

_CACHE = {}


def kernel(**inputs):
    x = np.asarray(inputs["x"], np.float32)
    edge_index = np.asarray(inputs["edge_index"])
    edge_attr = np.asarray(inputs["edge_attr"], np.float32)

    meta = _prep_graph(edge_index)
    params = _prep_params(inputs)
    core_inputs = _prep_core_inputs(meta, x, edge_attr, params)

    NT = meta["NT"]
    if NT not in _CACHE:
        nc = build_kernel(NT)
        nc.compile()
        _CACHE[NT] = nc
    nc = _CACHE[NT]

    res = run_bass_kernel_spmd(nc, core_inputs, core_ids=list(range(NCORES)))
    y = np.concatenate([res.results[r]["y"][0] for r in range(NCORES)])
    return y[:N].reshape(N, 1).astype(np.float32)


if __name__ == "__main__":
    import reference
    ins = {k: np.asarray(v) for k, v in reference.setup_inputs().items()}
    out = kernel(**ins)
    print(out.shape, out.dtype, out[:4, 0])


# revision 13
# speedup vs baseline: 1.0675x; 1.0675x over previous
"""GATNet (3-layer GAT with edge features) on 8 Trainium2 NeuronCores.

Strategy (dst-sharded, edge-sorted, host-assisted):
  - Nodes padded to N_PAD=20480 -> 160 chunks of 128 nodes; 20 chunks/core.
  - Edges + one self-loop per node, sorted by dst; every edge lands on the
    core owning its dst chunk => all segment softmax/aggregation core-local.
  - Host precomputes (not on the device critical path):
      * a_e = edge_attr @ (We folded with att_e) for all 3 layers, incl. the
        self-loop rows (PyG fill_value='mean' -> per-dst mean of real a_e)
        and -1e4 on padding slots (kills exp); staged as bf16 input.
      * one-hot scatter matrices M1 (edge->dst, fp8, SBUF-resident) and M1T
        (dst->edge, fp8, streamed per chunk) for the PE-matmul scatter/
        broadcast; identity I128 fp8.
      * layer-1 projections: hs1 table [h|a_s] (bf16, replicated) and a_d1.
  - Per layer: (L2/L3 only) cores project their own 2560-node shard with one
    fused matmul (att_s/att_d folded into W on host), AllGather the bf16
    [h|a_s] table; then per chunk:
      dma_gather h-rows by src; alpha = M1T@a_d + I@a_e + I@a_s accumulated
      on the PE into PSUM; DVE leaky-relu; Act engine exp with C-broadcast
      (L3 split Act/Pool); DVE 2x multiply exh = ex*h in place; PE matmuls
      M1^T @ [exh|ex] scatter-add numerator+denominator into PSUM; DVE
      divide+relu.  Softmax max-subtraction dropped (alpha is O(1)).
"""

import math
import sys

import numpy as np

sys.path.insert(0, "/opt/trn_rl_repo")

import ml_dtypes  # noqa: E402

import concourse.bacc as bacc  # noqa: E402
import concourse.bass as bass  # noqa: E402
import concourse.mybir as mybir  # noqa: E402
import concourse.tile as tile  # noqa: E402
from concourse.bass_utils import run_bass_kernel_spmd  # noqa: E402

bf16 = ml_dtypes.bfloat16
fp8 = ml_dtypes.float8_e3m4

N = 20000
E = 320000
FIN = 16
ED = 22
NEG = 0.2
NCORES = 8
NPAD = 20480
NCH = NPAD // 128 // NCORES     # 20 chunks per core
SHARD = NPAD // NCORES          # 2560 own nodes per core
LAYERS = [(16, 8, 32), (256, 8, 32), (256, 12, 64)]
ROWW = [384, 384, 896]          # hs row: [h(HC) | a_s(H) | pad], bytes%256==0
AEW = [8, 8, 12]
AE_TOT = 28
F32 = mybir.dt.float32
BF16 = mybir.dt.bfloat16
FP8 = mybir.dt.float8e3
I16 = mybir.dt.int16
AF = mybir.ActivationFunctionType
ALU = mybir.AluOpType
L3_ACT_FRAC = 0.62              # fraction of L3 ex-broadcast done on Act


# ============================ host-side prep ============================

def _prep_graph(edge_index):
    src = edge_index[0].astype(np.int64)
    dst = edge_index[1].astype(np.int64)
    order = np.argsort(dst, kind="stable")
    src_s = src[order]
    dst_s = dst[order]
    cnt = np.bincount(dst, minlength=NPAD)

    nchunks = NPAD // 128
    chunk_of = dst_s // 128
    chunk_cnt = np.bincount(chunk_of, minlength=nchunks) + 128
    NT = int(math.ceil(chunk_cnt.max() / 128.0))
    SL = NT * 128

    tot = nchunks * SL
    g_src = np.zeros(tot, np.int64)
    g_dstloc = np.zeros(tot, np.int64)
    g_isself = np.zeros(tot, np.bool_)
    g_eaidx = np.full(tot, -1, np.int64)

    starts = np.searchsorted(chunk_of, np.arange(nchunks))
    ends = np.searchsorted(chunk_of, np.arange(nchunks) + 1)
    for c in range(nchunks):
        base = c * SL
        r0, r1 = int(starts[c]), int(ends[c])
        d_loc = dst_s[r0:r1] - c * 128
        nreal = r1 - r0
        seg_cnt = np.bincount(d_loc, minlength=128)
        blk_off = np.zeros(128, np.int64)
        np.cumsum(seg_cnt[:-1] + 1, out=blk_off[1:])
        within = np.arange(nreal) - np.repeat(np.cumsum(seg_cnt) - seg_cnt, seg_cnt)
        pos = base + blk_off[d_loc] + within
        g_src[pos] = src_s[r0:r1]
        g_dstloc[pos] = d_loc
        g_eaidx[pos] = order[r0:r1]
        pos_self = base + blk_off + seg_cnt
        g_src[pos_self] = c * 128 + np.arange(128)
        g_dstloc[pos_self] = np.arange(128)
        g_isself[pos_self] = True

    g_ispad = (g_eaidx < 0) & ~g_isself
    return {"NT": NT, "SL": SL, "cnt": cnt, "src": g_src,
            "dstloc": g_dstloc, "isself": g_isself, "eaidx": g_eaidx,
            "ispad": g_ispad}


def _wrap_idx(idx):
    n = idx.shape[0]
    w = idx.astype(np.int16).reshape(n // 16, 16).T
    return np.tile(w, (8, 1))


def _prep_params(kw):
    p = {}
    wered = []
    for li, (fin, H, C) in enumerate(LAYERS):
        i = li + 1
        W = kw[f"W{i}"].astype(np.float32)
        We = kw[f"We{i}"].astype(np.float32)
        ats = kw[f"as{i}"].astype(np.float32)
        atd = kw[f"ad{i}"].astype(np.float32)
        ate = kw[f"ae{i}"].astype(np.float32)
        Was = np.einsum("dhc,hc->dh", W.reshape(fin, H, C), ats)
        Wad = np.einsum("dhc,hc->dh", W.reshape(fin, H, C), atd)
        if li > 0:
            p[f"Wall{i}"] = np.concatenate([W, Was, Wad], axis=1).astype(bf16)
            bias = np.zeros((1, H * C + H), np.float32)
            bias[0, :H * C] = kw[f"b{i}"].astype(np.float32)
            p[f"bias{i}"] = bias.astype(bf16)
        else:
            # layer 1 projections are host-computed from x directly
            p["bias1"] = np.concatenate(
                [kw["b1"].astype(np.float32), np.zeros(H, np.float32)]
            ).reshape(1, -1).astype(bf16)
            p["_W1"] = W
            p["_Was1"] = Was
            p["_Wad1"] = Wad
        wered.append(np.einsum("dhc,hc->dh", We.reshape(ED, H, C), ate))
    p["_wered"] = np.concatenate(wered, axis=1).astype(np.float32)  # [ED,28]
    Wf = kw["Wf"].astype(np.float32).reshape(-1)
    p["wf1"] = np.ascontiguousarray(Wf[0:256].reshape(2, 128).T).astype(bf16)
    p["wf2"] = np.ascontiguousarray(Wf[256:512].reshape(2, 128).T).astype(bf16)
    p["wf3"] = np.ascontiguousarray(Wf[512:1280].reshape(6, 128).T).astype(bf16)
    p["bf"] = kw["bf"].astype(np.float32).reshape(1, 1)
    p["ident"] = np.eye(128, dtype=fp8)
    return p


def _prep_core_inputs(meta, x, edge_attr, params):
    NT, SL = meta["NT"], meta["SL"]
    npc = NCH * SL

    # ---- layer-1 host projections ----
    x_pad = np.zeros((NPAD, FIN), np.float32)
    x_pad[:N] = x
    h1 = x_pad @ params["_W1"]                        # [NPAD, 256]
    as1 = x_pad @ params["_Was1"]                     # [NPAD, 8]
    ad1 = x_pad @ params["_Wad1"]                     # [NPAD, 8]
    hs1 = np.zeros((NPAD, ROWW[0]), np.float32)
    hs1[:, 0:256] = h1
    hs1[:, 256:264] = as1
    hs1_b = hs1.astype(bf16)

    # ---- a_e for all edge slots (3 layers fused), incl. self means/pad ----
    ae_edge = edge_attr.astype(np.float32) @ params["_wered"]   # [E, 28]
    cnt = np.maximum(meta["cnt"][:, None], 1.0)
    sums = np.zeros((NPAD, AE_TOT), np.float32)
    dst_full = np.zeros(0)
    # mean per dst over real incoming edges
    # (use eaidx/dstloc info: real slots have eaidx>=0)
    real = meta["eaidx"] >= 0
    slot_chunk = np.arange(160 * SL) // SL
    dst_node = slot_chunk * 128 + meta["dstloc"]
    np.add.at(sums, dst_node[real], ae_edge[meta["eaidx"][real]])
    mean_ae = sums / cnt
    ae_slot = np.zeros((160 * SL, AE_TOT), np.float32)
    ae_slot[real] = ae_edge[meta["eaidx"][real]]
    ae_slot[meta["isself"]] = mean_ae[dst_node[meta["isself"]]]
    ae_slot[meta["ispad"]] = -1e4
    # layer-1 a_d[dst] is host-known: fold it into layer-1 a_e
    ae_slot[:, 0:8] += ad1[dst_node]

    # ---- one-hot scatter matrices ----
    dl = meta["dstloc"].reshape(160, NT, 128)         # [chunk, t, e]
    eye = np.eye(128, dtype=fp8)
    m1_all = eye[dl]                                  # [chunk, t, e(part), n]
    ins = []
    for r in range(NCORES):
        sl = slice(r * npc, (r + 1) * npc)
        chs = slice(r * NCH, (r + 1) * NCH)
        idx16 = _wrap_idx(meta["src"][sl])
        # M1: [128(e), NCH*NT*128(n)]
        m1 = np.ascontiguousarray(
            m1_all[chs].transpose(2, 0, 1, 3).reshape(128, NCH * NT * 128))
        # M1T: [128(n), NCH*NT*128(e)]
        m1t = np.ascontiguousarray(
            m1_all[chs].transpose(3, 0, 1, 2).reshape(128, NCH * NT * 128))
        d = {"idx16": idx16, "m1": m1, "m1t": m1t, "hs1": hs1_b}
        off = 0
        for li, w in enumerate(AEW):
            ae_c = np.ascontiguousarray(
                ae_slot[sl, off:off + w].reshape(NCH * NT, 128, w)
                .transpose(1, 0, 2).reshape(128, NCH * NT * w)).astype(bf16)
            d[f"ae{li + 1}c"] = ae_c
            off += w
        d.update({k: v for k, v in params.items() if not k.startswith("_")})
        ins.append(d)
    return ins


# ============================ device kernel ============================

def build_kernel(NT, nch=NCH, use_cc=True):
    NCHl = nch
    SHARDl = NCHl * 128
    NPADl = SHARDl * NCORES
    SL = NT * 128
    npc = NCHl * SL
    TPC = NCHl * NT

    nc = bacc.Bacc("TRN2", num_devices=NCORES)

    d_idx = nc.dram_tensor("idx16", [128, npc // 16], I16, kind="ExternalInput")
    d_m1 = nc.dram_tensor("m1", [128, npc], FP8, kind="ExternalInput")
    d_m1t = nc.dram_tensor("m1t", [128, npc], FP8, kind="ExternalInput")
    d_aec = [nc.dram_tensor(f"ae{li + 1}c", [128, TPC * AEW[li]], BF16,
                            kind="ExternalInput") for li in range(3)]
    d_hs1 = nc.dram_tensor("hs1", [NPADl, ROWW[0]], BF16, kind="ExternalInput")
    d_ident = nc.dram_tensor("ident", [128, 128], FP8, kind="ExternalInput")
    d_Wall, d_bias = {}, {}
    for li, (fin, H, C) in enumerate(LAYERS):
        if li > 0:
            d_Wall[li] = nc.dram_tensor(f"Wall{li + 1}", [fin, H * C + 2 * H],
                                        BF16, kind="ExternalInput")
        d_bias[li] = nc.dram_tensor(f"bias{li + 1}", [1, H * C + H], BF16,
                                    kind="ExternalInput")
    d_wf = [nc.dram_tensor(f"wf{i + 1}", [128, nb], BF16, kind="ExternalInput")
            for i, nb in enumerate((2, 2, 6))]
    d_bf = nc.dram_tensor("bf", [1, 1], F32, kind="ExternalInput")
    d_y = nc.dram_tensor("y", [1, SHARDl], F32, kind="ExternalOutput")

    with tile.TileContext(nc) as tc:
        with tc.tile_pool(name="const", bufs=1) as cpool, \
             tc.tile_pool(name="lay", bufs=1) as lpool, \
             tc.tile_pool(name="work", bufs=2) as wpool, \
             tc.tile_pool(name="gbuf", bufs=2) as gpool, \
             tc.tile_pool(name="exbuf", bufs=2) as xpool, \
             tc.tile_pool(name="psbig", bufs=2, space="PSUM") as psb, \
             tc.tile_pool(name="pssm", bufs=2, space="PSUM") as pss:

            # internal DRAM
            d_hs_in = [None] + [nc.dram_tensor(f"d_hs_in{li}", [SHARDl, ROWW[li]],
                                               BF16) for li in (1, 2)]
            d_hs = [None] + [nc.dram_tensor(f"d_hs{li}", [NPADl, ROWW[li]], BF16)
                             for li in (1, 2)]
            d_x = [nc.dram_tensor(f"d_x{li}",
                                  [SHARDl, LAYERS[li][1] * LAYERS[li][2]], BF16)
                   for li in range(3)]

            # ---------- constants ----------
            t_ones = cpool.tile([1, 128], BF16)
            nc.vector.memset(t_ones[:], 1.0)
            t_ident = cpool.tile([128, 128], FP8)
            nc.sync.dma_start(out=t_ident[:], in_=d_ident[:])
            t_idx = cpool.tile([128, npc // 16], I16)
            nc.sync.dma_start(out=t_idx[:], in_=d_idx[:])
            t_m1 = cpool.tile([128, npc], FP8)
            nc.sync.dma_start(out=t_m1[:], in_=d_m1[:])

            # ---------- layers ----------
            for li, (fin, H, C) in enumerate(LAYERS):
                HC = H * C
                RW_ = ROWW[li]
                AEw = AE_TOT
                NDW = HC + H
                PJW = HC + 2 * H
                nkb = max(fin // 128, 1)
                KP = min(fin, 128)

                t_bias = lpool.tile([1, NDW], BF16, tag=f"bias{li}")
                nc.sync.dma_start(out=t_bias[:], in_=d_bias[li][:])
                t_ad = lpool.tile([128, NCHl * H], BF16, tag=f"ad{li}")

                # ---- phase A: own-shard projections -> hs shard + a_d ----
                if li == 0:
                    hs_tab = d_hs1
                else:
                    t_W = lpool.tile([KP, nkb, PJW], BF16, tag=f"W{li}")
                    nc.sync.dma_start(
                        out=t_W[:],
                        in_=d_Wall[li][:].rearrange("(b p) w -> p b w", p=KP))
                    t_xT = lpool.tile([128, nkb * SHARDl], BF16, tag=f"xT{li}")
                    for b in range(nkb):
                        nc.sync.dma_start(
                            out=t_xT[:, b * SHARDl:(b + 1) * SHARDl],
                            in_=d_x[li - 1][:, b * 128:(b + 1) * 128],
                            transpose=True)
                    for ch in range(NCHl):
                        p_h = psb.tile([128, 1024], F32, space="PSUM", tag="big")
                        for b in range(nkb):
                            xsl = t_xT[:KP, b * SHARDl + ch * 128:
                                       b * SHARDl + ch * 128 + 128]
                            for c0 in range(0, PJW, 512):
                                c1 = min(c0 + 512, PJW)
                                nc.tensor.matmul(
                                    out=p_h[:, c0:c1],
                                    lhsT=xsl,
                                    rhs=t_W[:, b, c0:c1],
                                    start=(b == 0), stop=(b == nkb - 1))
                        t_hs = wpool.tile([128, RW_], BF16, tag="hsrow")
                        nc.scalar.copy(out=t_hs[:, 0:HC + H], in_=p_h[:, 0:HC + H])
                        nc.sync.dma_start(out=d_hs_in[li][ch * 128:(ch + 1) * 128, :],
                                          in_=t_hs[:])
                        nc.vector.tensor_copy(
                            t_ad[:, ch * H:(ch + 1) * H],
                            p_h[:, HC + H:HC + 2 * H])

                    if use_cc:
                        nc.gpsimd.collective_compute(
                            "AllGather", ALU.bypass,
                            replica_groups=[list(range(NCORES))],
                            ins=[d_hs_in[li].ap().opt()],
                            outs=[d_hs[li].ap().opt()])
                    else:
                        nc.gpsimd.dma_start(out=d_hs[li][0:SHARDl, :],
                                            in_=d_hs_in[li][:])
                    hs_tab = d_hs[li]

                # ---- phase B: edges ----
                for ch in range(NCHl):
                    t_g = gpool.tile([128, NT, RW_], BF16, tag="G")
                    nc.gpsimd.dma_gather(t_g[:], hs_tab[:],
                                         t_idx[:, ch * SL // 16:(ch + 1) * SL // 16],
                                         SL, SL, RW_, single_packet=False)
                    AEw_l = AEW[li]
                    t_ae = wpool.tile([128, NT, AEw_l], BF16, tag="aeL")
                    nc.sync.dma_start(
                        out=t_ae[:],
                        in_=d_aec[li][:, ch * NT * AEw_l:(ch + 1) * NT * AEw_l]
                        .rearrange("p (t h) -> p t h", t=NT))
                    if li > 0:
                        t_m1t = wpool.tile([128, SL], FP8, tag="m1t")
                        nc.sync.dma_start(out=t_m1t[:],
                                          in_=d_m1t[:, ch * SL:(ch + 1) * SL])

                    # three passes over half-chunks so each engine's in-order
                    # queue never parks behind a cross-engine wait:
                    #   pass 1 (PE): alpha = a_d[dst] + a_e + a_s[src]
                    #   pass 2 (DVE/Act/Pool): lrelu -> exp + C-replicate
                    #   pass 3 (DVE then PE): exh multiply -> scatter matmuls
                    NH = (NT + 1) // 2
                    halves = [(0, NH), (NH, NT)]
                    p_als, t_lrs, t_exCs = [], [], []
                    for h0, h1 in halves:
                        nh = h1 - h0
                        p_al = pss.tile([128, NH, H], F32, space="PSUM",
                                        tag="psmA")
                        p_als.append(p_al)
                        for t in range(h0, h1):
                            tt = t - h0
                            if li > 0:
                                nc.tensor.matmul(
                                    out=p_al[:, tt, :],
                                    lhsT=t_m1t[:, t * 128:(t + 1) * 128],
                                    rhs=t_ad[:, ch * H:(ch + 1) * H],
                                    start=True, stop=False)
                            nc.tensor.matmul(out=p_al[:, tt, :],
                                             lhsT=t_ident[:],
                                             rhs=t_ae[:, t, :],
                                             start=(li == 0), stop=False)
                            nc.tensor.matmul(out=p_al[:, tt, :],
                                             lhsT=t_ident[:],
                                             rhs=t_g[:, t, HC:HC + H],
                                             start=False, stop=True)
                    for hi, (h0, h1) in enumerate(halves):
                        nh = h1 - h0
                        p_al = p_als[hi]
                        t_al = wpool.tile([128, NH, H], F32, tag="al")
                        nc.vector.tensor_copy(t_al[:, 0:nh], p_al[:, 0:nh])
                        t_lr = wpool.tile([128, NH, H], F32, tag="lr")
                        nc.vector.scalar_tensor_tensor(
                            out=t_lr[:, 0:nh], in0=t_al[:, 0:nh], scalar=NEG,
                            in1=t_al[:, 0:nh], op0=ALU.mult, op1=ALU.max)
                        t_exC = xpool.tile([128, NH, H, C], BF16, tag="exC")
                        t_exCs.append(t_exC)
                        if li < 2:
                            nc.scalar.activation(
                                t_exC[:, 0:nh],
                                t_lr[:, 0:nh].unsqueeze(-1)
                                .broadcast_to([128, nh, H, C]),
                                AF.Exp)
                        else:
                            t_ex = wpool.tile([128, NH, H], BF16, tag="ex")
                            nc.scalar.activation(t_ex[:, 0:nh], t_lr[:, 0:nh],
                                                 AF.Exp)
                            ta = max(1, int(nh * L3_ACT_FRAC))
                            nc.scalar.copy(
                                out=t_exC[:, 0:ta],
                                in_=t_ex[:, 0:ta].unsqueeze(-1)
                                .broadcast_to([128, ta, H, C]))
                            nc.gpsimd.tensor_copy(
                                t_exC[:, ta:nh],
                                t_ex[:, ta:nh].unsqueeze(-1)
                                .broadcast_to([128, nh - ta, H, C]))

                    p_nd = psb.tile([128, 1024], F32, space="PSUM", tag="big")
                    for c0 in range(0, NDW, 512):
                        c1 = min(c0 + 512, NDW)
                        nc.tensor.matmul(out=p_nd[:, c0:c1], lhsT=t_ones[:],
                                         rhs=t_bias[:, c0:c1], start=True,
                                         stop=False)
                    m1base = ch * SL
                    for hi, (h0, h1) in enumerate(halves):
                        nh = h1 - h0
                        t_exC = t_exCs[hi]
                        # exh = ex*h in place (2x mode: all bf16 packed)
                        nc.vector.tensor_tensor(
                            out=t_g[:, h0:h1, 0:HC],
                            in0=t_g[:, h0:h1, 0:HC],
                            in1=t_exC[:, 0:nh].rearrange("p t h c -> p t (h c)"),
                            op=ALU.mult)
                        # denominator cols: ex (pick c=0 stride-C view)
                        nc.vector.tensor_copy(t_g[:, h0:h1, HC:NDW],
                                              t_exC[:, 0:nh, :, 0])
                        # scatter this half: NUMDEN += sum_t M1_t.T @ exh_t
                        for t in range(h0, h1):
                            for c0 in range(0, NDW, 512):
                                c1 = min(c0 + 512, NDW)
                                nc.tensor.matmul(
                                    out=p_nd[:, c0:c1],
                                    lhsT=t_m1[:, m1base + t * 128:
                                              m1base + (t + 1) * 128],
                                    rhs=t_g[:, t, c0:c1],
                                    start=False, stop=(t == NT - 1))
                    # x = relu(num/den)
                    t_rec = wpool.tile([128, H], F32, tag="rec")
                    nc.vector.reciprocal(t_rec[:], p_nd[:, HC:NDW])
                    t_x = wpool.tile([128, HC], BF16, tag="xout")
                    nc.vector.scalar_tensor_tensor(
                        out=t_x[:].rearrange("p (h c) -> p h c", h=H),
                        in0=p_nd[:, 0:HC].rearrange("p (h c) -> p h c", h=H),
                        scalar=0.0, op0=ALU.max, op1=ALU.mult,
                        in1=t_rec[:].unsqueeze(-1).broadcast_to([128, H, C]))
                    nc.sync.dma_start(out=d_x[li][ch * 128:(ch + 1) * 128, :],
                                      in_=t_x[:])

            # ---------- final: y = sigmoid(concat(x1,x2,x3) @ Wf + bf) ----------
            t_wf = [lpool.tile([128, nb], BF16, tag=f"wf{i}", name=f"t_wf{i}")
                    for i, nb in enumerate((2, 2, 6))]
            for i in range(3):
                nc.sync.dma_start(out=t_wf[i][:], in_=d_wf[i][:])
            t_bf = lpool.tile([1, 1], F32, tag="bf")
            nc.sync.dma_start(out=t_bf[:], in_=d_bf[:])
            for g in range(SHARDl // 512):
                p_y = pss.tile([1, 512], F32, space="PSUM", tag="psmB")
                first = True
                for li in range(3):
                    nbl = (LAYERS[li][1] * LAYERS[li][2]) // 128
                    for b in range(nbl):
                        t_xg = wpool.tile([128, 512], BF16, tag="xg")
                        nc.sync.dma_start(
                            out=t_xg[:],
                            in_=d_x[li][g * 512:(g + 1) * 512,
                                        b * 128:(b + 1) * 128],
                            transpose=True)
                        nc.tensor.matmul(out=p_y[:], lhsT=t_wf[li][:, b:b + 1],
                                         rhs=t_xg[:], start=first,
                                         stop=(li == 2 and b == nbl - 1))
                        first = False
                t_y = wpool.tile([1, 512], F32, tag="yrow")
                nc.scalar.activation(t_y[:], p_y[:], AF.Sigmoid, bias=t_bf[:])
                nc.sync.dma_start(out=d_y[0:1, g * 512:(g + 1) * 512], in_=t_y[:])

    return nc


# ============================ public entry ============================

_CACHE = {}


def kernel(**inputs):
    x = np.asarray(inputs["x"], np.float32)
    edge_index = np.asarray(inputs["edge_index"])
    edge_attr = np.asarray(inputs["edge_attr"], np.float32)

    meta = _prep_graph(edge_index)
    params = _prep_params(inputs)
    core_inputs = _prep_core_inputs(meta, x, edge_attr, params)

    NT = meta["NT"]
    if NT not in _CACHE:
        nc = build_kernel(NT)
        nc.compile()
        _CACHE[NT] = nc
    nc = _CACHE[NT]

    res = run_bass_kernel_spmd(nc, core_inputs, core_ids=list(range(NCORES)))
    y = np.concatenate([res.results[r]["y"][0] for r in range(NCORES)])
    return y[:N].reshape(N, 1).astype(np.float32)


if __name__ == "__main__":
    import reference
    ins = {k: np.asarray(v) for k, v in reference.setup_inputs().items()}
    out = kernel(**ins)
    print(out.shape, out.dtype, out[:4, 0])


# revision 14
# speedup vs baseline: 1.2480x; 1.1691x over previous
"""GATNet (3-layer GAT with edge features) on 8 Trainium2 NeuronCores.

Strategy (dst-sharded, edge-sorted, host-assisted):
  - Nodes padded to N_PAD=20480 -> 160 chunks of 128 nodes; 20 chunks/core.
  - Edges + one self-loop per node, sorted by dst; every edge lands on the
    core owning its dst chunk => all segment softmax/aggregation core-local.
  - Host precomputes (not on the device critical path):
      * a_e = edge_attr @ (We folded with att_e) for all 3 layers, incl. the
        self-loop rows (PyG fill_value='mean' -> per-dst mean of real a_e)
        and -1e4 on padding slots (kills exp); staged as bf16 input.
      * one-hot scatter matrices M1 (edge->dst, fp8, SBUF-resident) and M1T
        (dst->edge, fp8, streamed per chunk) for the PE-matmul scatter/
        broadcast; identity I128 fp8.
      * layer-1 projections: hs1 table [h|a_s] (bf16, replicated) and a_d1.
  - Per layer: (L2/L3 only) cores project their own 2560-node shard with one
    fused matmul (att_s/att_d folded into W on host), AllGather the bf16
    [h|a_s] table; then per chunk:
      dma_gather h-rows by src; alpha = M1T@a_d + I@a_e + I@a_s accumulated
      on the PE into PSUM; DVE leaky-relu; Act engine exp with C-broadcast
      (L3 split Act/Pool); DVE 2x multiply exh = ex*h in place; PE matmuls
      M1^T @ [exh|ex] scatter-add numerator+denominator into PSUM; DVE
      divide+relu.  Softmax max-subtraction dropped (alpha is O(1)).
"""

import math
import sys

import numpy as np

sys.path.insert(0, "/opt/trn_rl_repo")

import ml_dtypes  # noqa: E402

import concourse.bacc as bacc  # noqa: E402
import concourse.bass as bass  # noqa: E402
import concourse.mybir as mybir  # noqa: E402
import concourse.tile as tile  # noqa: E402
from concourse.bass_utils import run_bass_kernel_spmd  # noqa: E402

bf16 = ml_dtypes.bfloat16
fp8 = ml_dtypes.float8_e3m4

N = 20000
E = 320000
FIN = 16
ED = 22
NEG = 0.2
NCORES = 8
NPAD = 20480
NCH = NPAD // 128 // NCORES     # 20 chunks per core
SHARD = NPAD // NCORES          # 2560 own nodes per core
LAYERS = [(16, 8, 32), (256, 8, 32), (256, 12, 64)]
ROWW = [384, 384, 896]          # hs row: [h(HC) | a_s(H) | pad], bytes%256==0
AEW = [8, 8, 12]
AE_TOT = 28
F32 = mybir.dt.float32
BF16 = mybir.dt.bfloat16
FP8 = mybir.dt.float8e3
I16 = mybir.dt.int16
AF = mybir.ActivationFunctionType
ALU = mybir.AluOpType
L3_ACT_FRAC = 0.62              # fraction of L3 ex-broadcast done on Act


# ============================ host-side prep ============================

def _prep_graph(edge_index):
    src = edge_index[0].astype(np.int64)
    dst = edge_index[1].astype(np.int64)
    order = np.argsort(dst, kind="stable")
    src_s = src[order]
    dst_s = dst[order]
    cnt = np.bincount(dst, minlength=NPAD)

    nchunks = NPAD // 128
    chunk_of = dst_s // 128
    chunk_cnt = np.bincount(chunk_of, minlength=nchunks) + 128
    NT = int(math.ceil(chunk_cnt.max() / 128.0))
    SL = NT * 128

    tot = nchunks * SL
    g_src = np.zeros(tot, np.int64)
    g_dstloc = np.zeros(tot, np.int64)
    g_isself = np.zeros(tot, np.bool_)
    g_eaidx = np.full(tot, -1, np.int64)

    starts = np.searchsorted(chunk_of, np.arange(nchunks))
    ends = np.searchsorted(chunk_of, np.arange(nchunks) + 1)
    for c in range(nchunks):
        base = c * SL
        r0, r1 = int(starts[c]), int(ends[c])
        d_loc = dst_s[r0:r1] - c * 128
        nreal = r1 - r0
        seg_cnt = np.bincount(d_loc, minlength=128)
        blk_off = np.zeros(128, np.int64)
        np.cumsum(seg_cnt[:-1] + 1, out=blk_off[1:])
        within = np.arange(nreal) - np.repeat(np.cumsum(seg_cnt) - seg_cnt, seg_cnt)
        pos = base + blk_off[d_loc] + within
        g_src[pos] = src_s[r0:r1]
        g_dstloc[pos] = d_loc
        g_eaidx[pos] = order[r0:r1]
        pos_self = base + blk_off + seg_cnt
        g_src[pos_self] = c * 128 + np.arange(128)
        g_dstloc[pos_self] = np.arange(128)
        g_isself[pos_self] = True

    g_ispad = (g_eaidx < 0) & ~g_isself
    return {"NT": NT, "SL": SL, "cnt": cnt, "src": g_src,
            "dstloc": g_dstloc, "isself": g_isself, "eaidx": g_eaidx,
            "ispad": g_ispad}


def _wrap_idx(idx):
    n = idx.shape[0]
    w = idx.astype(np.int16).reshape(n // 16, 16).T
    return np.tile(w, (8, 1))


def _prep_params(kw):
    p = {}
    wered = []
    for li, (fin, H, C) in enumerate(LAYERS):
        i = li + 1
        W = kw[f"W{i}"].astype(np.float32)
        We = kw[f"We{i}"].astype(np.float32)
        ats = kw[f"as{i}"].astype(np.float32)
        atd = kw[f"ad{i}"].astype(np.float32)
        ate = kw[f"ae{i}"].astype(np.float32)
        Was = np.einsum("dhc,hc->dh", W.reshape(fin, H, C), ats)
        Wad = np.einsum("dhc,hc->dh", W.reshape(fin, H, C), atd)
        if li > 0:
            p[f"Wall{i}"] = np.concatenate([W, Was, Wad], axis=1).astype(bf16)
            bias = np.zeros((1, H * C + H), np.float32)
            bias[0, :H * C] = kw[f"b{i}"].astype(np.float32)
            p[f"bias{i}"] = bias.astype(bf16)
        else:
            # layer 1 projections are host-computed from x directly
            p["bias1"] = np.concatenate(
                [kw["b1"].astype(np.float32), np.zeros(H, np.float32)]
            ).reshape(1, -1).astype(bf16)
            p["_W1"] = W
            p["_Was1"] = Was
            p["_Wad1"] = Wad
        wered.append(np.einsum("dhc,hc->dh", We.reshape(ED, H, C), ate))
    p["_wered"] = np.concatenate(wered, axis=1).astype(np.float32)  # [ED,28]
    Wf = kw["Wf"].astype(np.float32).reshape(-1)
    p["wf1"] = np.ascontiguousarray(Wf[0:256].reshape(2, 128).T).astype(bf16)
    p["wf2"] = np.ascontiguousarray(Wf[256:512].reshape(2, 128).T).astype(bf16)
    p["wf3"] = np.ascontiguousarray(Wf[512:1280].reshape(6, 128).T).astype(bf16)
    p["bf"] = kw["bf"].astype(np.float32).reshape(1, 1)
    p["ident"] = np.eye(128, dtype=fp8)
    return p


def _prep_core_inputs(meta, x, edge_attr, params):
    NT, SL = meta["NT"], meta["SL"]
    npc = NCH * SL

    # ---- layer-1 host projections ----
    x_pad = np.zeros((NPAD, FIN), np.float32)
    x_pad[:N] = x
    h1 = x_pad @ params["_W1"]                        # [NPAD, 256]
    as1 = x_pad @ params["_Was1"]                     # [NPAD, 8]
    ad1 = x_pad @ params["_Wad1"]                     # [NPAD, 8]
    hs1 = np.zeros((NPAD, ROWW[0]), np.float32)
    hs1[:, 0:256] = h1
    hs1[:, 256:264] = as1
    hs1_b = hs1.astype(bf16)

    # ---- a_e for all edge slots (3 layers fused), incl. self means/pad ----
    ae_edge = edge_attr.astype(np.float32) @ params["_wered"]   # [E, 28]
    cnt = np.maximum(meta["cnt"][:, None], 1.0)
    sums = np.zeros((NPAD, AE_TOT), np.float32)
    dst_full = np.zeros(0)
    # mean per dst over real incoming edges
    # (use eaidx/dstloc info: real slots have eaidx>=0)
    real = meta["eaidx"] >= 0
    slot_chunk = np.arange(160 * SL) // SL
    dst_node = slot_chunk * 128 + meta["dstloc"]
    np.add.at(sums, dst_node[real], ae_edge[meta["eaidx"][real]])
    mean_ae = sums / cnt
    ae_slot = np.zeros((160 * SL, AE_TOT), np.float32)
    ae_slot[real] = ae_edge[meta["eaidx"][real]]
    ae_slot[meta["isself"]] = mean_ae[dst_node[meta["isself"]]]
    ae_slot[meta["ispad"]] = -1e4
    # layer-1 a_d[dst] is host-known: fold it into layer-1 a_e
    ae_slot[:, 0:8] += ad1[dst_node]

    # ---- one-hot scatter matrices ----
    dl = meta["dstloc"].reshape(160, NT, 128)         # [chunk, t, e]
    eye = np.eye(128, dtype=fp8)
    m1_all = eye[dl]                                  # [chunk, t, e(part), n]
    ins = []
    for r in range(NCORES):
        sl = slice(r * npc, (r + 1) * npc)
        chs = slice(r * NCH, (r + 1) * NCH)
        idx16 = _wrap_idx(meta["src"][sl])
        # M1: [128(e), NCH*NT*128(n)]
        m1 = np.ascontiguousarray(
            m1_all[chs].transpose(2, 0, 1, 3).reshape(128, NCH * NT * 128))
        # M1T: [128(n), NCH*NT*128(e)]
        m1t = np.ascontiguousarray(
            m1_all[chs].transpose(3, 0, 1, 2).reshape(128, NCH * NT * 128))
        d = {"idx16": idx16, "m1": m1, "m1t": m1t, "hs1": hs1_b}
        off = 0
        for li, w in enumerate(AEW):
            ae_c = np.ascontiguousarray(
                ae_slot[sl, off:off + w].reshape(NCH * NT, 128, w)
                .transpose(1, 0, 2).reshape(128, NCH * NT * w)).astype(bf16)
            d[f"ae{li + 1}c"] = ae_c
            off += w
        d.update({k: v for k, v in params.items() if not k.startswith("_")})
        ins.append(d)
    return ins


# ============================ device kernel ============================

def build_kernel(NT, nch=NCH, use_cc=True):
    NCHl = nch
    SHARDl = NCHl * 128
    NPADl = SHARDl * NCORES
    SL = NT * 128
    npc = NCHl * SL
    TPC = NCHl * NT

    nc = bacc.Bacc("TRN2", num_devices=NCORES)

    d_idx = nc.dram_tensor("idx16", [128, npc // 16], I16, kind="ExternalInput")
    d_m1 = nc.dram_tensor("m1", [128, npc], FP8, kind="ExternalInput")
    d_m1t = nc.dram_tensor("m1t", [128, npc], FP8, kind="ExternalInput")
    d_aec = [nc.dram_tensor(f"ae{li + 1}c", [128, TPC * AEW[li]], BF16,
                            kind="ExternalInput") for li in range(3)]
    d_hs1 = nc.dram_tensor("hs1", [NPADl, ROWW[0]], BF16, kind="ExternalInput")
    d_ident = nc.dram_tensor("ident", [128, 128], FP8, kind="ExternalInput")
    d_Wall, d_bias = {}, {}
    for li, (fin, H, C) in enumerate(LAYERS):
        if li > 0:
            d_Wall[li] = nc.dram_tensor(f"Wall{li + 1}", [fin, H * C + 2 * H],
                                        BF16, kind="ExternalInput")
        d_bias[li] = nc.dram_tensor(f"bias{li + 1}", [1, H * C + H], BF16,
                                    kind="ExternalInput")
    d_wf = [nc.dram_tensor(f"wf{i + 1}", [128, nb], BF16, kind="ExternalInput")
            for i, nb in enumerate((2, 2, 6))]
    d_bf = nc.dram_tensor("bf", [1, 1], F32, kind="ExternalInput")
    d_y = nc.dram_tensor("y", [1, SHARDl], F32, kind="ExternalOutput")

    with tile.TileContext(nc) as tc:
        with tc.tile_pool(name="const", bufs=1) as cpool, \
             tc.tile_pool(name="lay", bufs=1) as lpool, \
             tc.tile_pool(name="work", bufs=2) as wpool, \
             tc.tile_pool(name="gbuf", bufs=3) as gpool, \
             tc.tile_pool(name="exbuf", bufs=2) as xpool, \
             tc.tile_pool(name="psbig", bufs=2, space="PSUM") as psb, \
             tc.tile_pool(name="pssm", bufs=2, space="PSUM") as pss:

            # internal DRAM
            d_hs_in = [None] + [nc.dram_tensor(f"d_hs_in{li}", [SHARDl, ROWW[li]],
                                               BF16) for li in (1, 2)]
            d_hs = [None] + [nc.dram_tensor(f"d_hs{li}", [NPADl, ROWW[li]], BF16)
                             for li in (1, 2)]
            d_x = [nc.dram_tensor(f"d_x{li}",
                                  [SHARDl, LAYERS[li][1] * LAYERS[li][2]], BF16)
                   for li in range(3)]

            # ---------- constants ----------
            t_ones = cpool.tile([1, 128], BF16)
            nc.vector.memset(t_ones[:], 1.0)
            t_ident = cpool.tile([128, 128], FP8)
            nc.sync.dma_start(out=t_ident[:], in_=d_ident[:])
            t_idx = cpool.tile([128, npc // 16], I16)
            nc.sync.dma_start(out=t_idx[:], in_=d_idx[:])

            # ---------- layers ----------
            for li, (fin, H, C) in enumerate(LAYERS):
                HC = H * C
                RW_ = ROWW[li]
                AEw = AE_TOT
                NDW = HC + H
                PJW = HC + 2 * H
                nkb = max(fin // 128, 1)
                KP = min(fin, 128)

                t_bias = lpool.tile([1, NDW], BF16, tag=f"bias{li}")
                nc.sync.dma_start(out=t_bias[:], in_=d_bias[li][:])
                t_ad = lpool.tile([128, NCHl * H], BF16, tag=f"ad{li}")

                # ---- phase A: own-shard projections -> hs shard + a_d ----
                if li == 0:
                    hs_tab = d_hs1
                else:
                    t_W = lpool.tile([KP, nkb, PJW], BF16, tag=f"W{li}")
                    nc.sync.dma_start(
                        out=t_W[:],
                        in_=d_Wall[li][:].rearrange("(b p) w -> p b w", p=KP))
                    t_xT = lpool.tile([128, nkb * SHARDl], BF16, tag=f"xT{li}")
                    for b in range(nkb):
                        nc.sync.dma_start(
                            out=t_xT[:, b * SHARDl:(b + 1) * SHARDl],
                            in_=d_x[li - 1][:, b * 128:(b + 1) * 128],
                            transpose=True)
                    for ch in range(NCHl):
                        p_h = psb.tile([128, 1024], F32, space="PSUM", tag="big")
                        for b in range(nkb):
                            xsl = t_xT[:KP, b * SHARDl + ch * 128:
                                       b * SHARDl + ch * 128 + 128]
                            for c0 in range(0, PJW, 512):
                                c1 = min(c0 + 512, PJW)
                                nc.tensor.matmul(
                                    out=p_h[:, c0:c1],
                                    lhsT=xsl,
                                    rhs=t_W[:, b, c0:c1],
                                    start=(b == 0), stop=(b == nkb - 1))
                        t_hs = wpool.tile([128, RW_], BF16, tag="hsrow")
                        nc.scalar.copy(out=t_hs[:, 0:HC + H], in_=p_h[:, 0:HC + H])
                        nc.sync.dma_start(out=d_hs_in[li][ch * 128:(ch + 1) * 128, :],
                                          in_=t_hs[:])
                        nc.vector.tensor_copy(
                            t_ad[:, ch * H:(ch + 1) * H],
                            p_h[:, HC + H:HC + 2 * H])

                    if use_cc:
                        nc.gpsimd.collective_compute(
                            "AllGather", ALU.bypass,
                            replica_groups=[list(range(NCORES))],
                            ins=[d_hs_in[li].ap().opt()],
                            outs=[d_hs[li].ap().opt()])
                    else:
                        nc.gpsimd.dma_start(out=d_hs[li][0:SHARDl, :],
                                            in_=d_hs_in[li][:])
                    hs_tab = d_hs[li]

                # ---- phase B: edges ----
                for ch in range(NCHl):
                    t_g = gpool.tile([128, NT, RW_], BF16, tag="G")
                    nc.gpsimd.dma_gather(t_g[:], hs_tab[:],
                                         t_idx[:, ch * SL // 16:(ch + 1) * SL // 16],
                                         SL, SL, RW_, single_packet=False)
                    AEw_l = AEW[li]
                    t_ae = wpool.tile([128, NT, AEw_l], BF16, tag="aeL")
                    nc.sync.dma_start(
                        out=t_ae[:],
                        in_=d_aec[li][:, ch * NT * AEw_l:(ch + 1) * NT * AEw_l]
                        .rearrange("p (t h) -> p t h", t=NT))
                    if li > 0:
                        t_m1t = wpool.tile([128, SL], FP8, tag="m1t")
                        nc.sync.dma_start(out=t_m1t[:],
                                          in_=d_m1t[:, ch * SL:(ch + 1) * SL])
                    t_m1 = wpool.tile([128, SL], FP8, tag="m1c")
                    nc.sync.dma_start(out=t_m1[:],
                                      in_=d_m1[:, ch * SL:(ch + 1) * SL])

                    # three passes over half-chunks so each engine's in-order
                    # queue never parks behind a cross-engine wait:
                    #   pass 1 (PE): alpha = a_d[dst] + a_e + a_s[src]
                    #   pass 2 (DVE/Act/Pool): lrelu -> exp + C-replicate
                    #   pass 3 (DVE then PE): exh multiply -> scatter matmuls
                    NH = (NT + 1) // 2
                    halves = [(0, NH), (NH, NT)]
                    p_als, t_lrs, t_exCs = [], [], []
                    for h0, h1 in halves:
                        nh = h1 - h0
                        p_al = pss.tile([128, NH, H], F32, space="PSUM",
                                        tag="psmA")
                        p_als.append(p_al)
                        for t in range(h0, h1):
                            tt = t - h0
                            if li > 0:
                                nc.tensor.matmul(
                                    out=p_al[:, tt, :],
                                    lhsT=t_m1t[:, t * 128:(t + 1) * 128],
                                    rhs=t_ad[:, ch * H:(ch + 1) * H],
                                    start=True, stop=False)
                            nc.tensor.matmul(out=p_al[:, tt, :],
                                             lhsT=t_ident[:],
                                             rhs=t_ae[:, t, :],
                                             start=(li == 0), stop=False)
                            nc.tensor.matmul(out=p_al[:, tt, :],
                                             lhsT=t_ident[:],
                                             rhs=t_g[:, t, HC:HC + H],
                                             start=False, stop=True)
                    for hi, (h0, h1) in enumerate(halves):
                        nh = h1 - h0
                        p_al = p_als[hi]
                        t_al = wpool.tile([128, NH, H], F32, tag="al")
                        nc.vector.tensor_copy(t_al[:, 0:nh], p_al[:, 0:nh])
                        t_lr = wpool.tile([128, NH, H], F32, tag="lr")
                        nc.vector.scalar_tensor_tensor(
                            out=t_lr[:, 0:nh], in0=t_al[:, 0:nh], scalar=NEG,
                            in1=t_al[:, 0:nh], op0=ALU.mult, op1=ALU.max)
                        t_exC = xpool.tile([128, NH, H, C], BF16, tag="exC")
                        t_exCs.append(t_exC)
                        if li < 2:
                            nc.scalar.activation(
                                t_exC[:, 0:nh],
                                t_lr[:, 0:nh].unsqueeze(-1)
                                .broadcast_to([128, nh, H, C]),
                                AF.Exp)
                        else:
                            t_ex = wpool.tile([128, NH, H], BF16, tag="ex")
                            nc.scalar.activation(t_ex[:, 0:nh], t_lr[:, 0:nh],
                                                 AF.Exp)
                            ta = max(1, int(nh * L3_ACT_FRAC))
                            nc.scalar.copy(
                                out=t_exC[:, 0:ta],
                                in_=t_ex[:, 0:ta].unsqueeze(-1)
                                .broadcast_to([128, ta, H, C]))
                            nc.gpsimd.tensor_copy(
                                t_exC[:, ta:nh],
                                t_ex[:, ta:nh].unsqueeze(-1)
                                .broadcast_to([128, nh - ta, H, C]))

                    p_nd = psb.tile([128, 1024], F32, space="PSUM", tag="big")
                    for c0 in range(0, NDW, 512):
                        c1 = min(c0 + 512, NDW)
                        nc.tensor.matmul(out=p_nd[:, c0:c1], lhsT=t_ones[:],
                                         rhs=t_bias[:, c0:c1], start=True,
                                         stop=False)
                    for hi, (h0, h1) in enumerate(halves):
                        nh = h1 - h0
                        t_exC = t_exCs[hi]
                        # exh = ex*h in place (2x mode: all bf16 packed)
                        nc.vector.tensor_tensor(
                            out=t_g[:, h0:h1, 0:HC],
                            in0=t_g[:, h0:h1, 0:HC],
                            in1=t_exC[:, 0:nh].rearrange("p t h c -> p t (h c)"),
                            op=ALU.mult)
                        # denominator cols: ex (pick c=0 stride-C view)
                        nc.vector.tensor_copy(t_g[:, h0:h1, HC:NDW],
                                              t_exC[:, 0:nh, :, 0])
                        # scatter this half: NUMDEN += sum_t M1_t.T @ exh_t
                        for t in range(h0, h1):
                            for c0 in range(0, NDW, 512):
                                c1 = min(c0 + 512, NDW)
                                nc.tensor.matmul(
                                    out=p_nd[:, c0:c1],
                                    lhsT=t_m1[:, t * 128:(t + 1) * 128],
                                    rhs=t_g[:, t, c0:c1],
                                    start=False, stop=(t == NT - 1))
                    # x = relu(num/den)
                    t_rec = wpool.tile([128, H], F32, tag="rec")
                    nc.vector.reciprocal(t_rec[:], p_nd[:, HC:NDW])
                    t_x = wpool.tile([128, HC], BF16, tag="xout")
                    nc.vector.scalar_tensor_tensor(
                        out=t_x[:].rearrange("p (h c) -> p h c", h=H),
                        in0=p_nd[:, 0:HC].rearrange("p (h c) -> p h c", h=H),
                        scalar=0.0, op0=ALU.max, op1=ALU.mult,
                        in1=t_rec[:].unsqueeze(-1).broadcast_to([128, H, C]))
                    nc.sync.dma_start(out=d_x[li][ch * 128:(ch + 1) * 128, :],
                                      in_=t_x[:])

            # ---------- final: y = sigmoid(concat(x1,x2,x3) @ Wf + bf) ----------
            t_wf = [lpool.tile([128, nb], BF16, tag=f"wf{i}", name=f"t_wf{i}")
                    for i, nb in enumerate((2, 2, 6))]
            for i in range(3):
                nc.sync.dma_start(out=t_wf[i][:], in_=d_wf[i][:])
            t_bf = lpool.tile([1, 1], F32, tag="bf")
            nc.sync.dma_start(out=t_bf[:], in_=d_bf[:])
            for g in range(SHARDl // 512):
                p_y = pss.tile([1, 512], F32, space="PSUM", tag="psmB")
                first = True
                for li in range(3):
                    nbl = (LAYERS[li][1] * LAYERS[li][2]) // 128
                    for b in range(nbl):
                        t_xg = wpool.tile([128, 512], BF16, tag="xg")
                        nc.sync.dma_start(
                            out=t_xg[:],
                            in_=d_x[li][g * 512:(g + 1) * 512,
                                        b * 128:(b + 1) * 128],
                            transpose=True)
                        nc.tensor.matmul(out=p_y[:], lhsT=t_wf[li][:, b:b + 1],
                                         rhs=t_xg[:], start=first,
                                         stop=(li == 2 and b == nbl - 1))
                        first = False
                t_y = wpool.tile([1, 512], F32, tag="yrow")
                nc.scalar.activation(t_y[:], p_y[:], AF.Sigmoid, bias=t_bf[:])
                nc.sync.dma_start(out=d_y[0:1, g * 512:(g + 1) * 512], in_=t_y[:])

    return nc


# ============================ public entry ============================

_CACHE = {}


def kernel(**inputs):
    x = np.asarray(inputs["x"], np.float32)
    edge_index = np.asarray(inputs["edge_index"])
    edge_attr = np.asarray(inputs["edge_attr"], np.float32)

    meta = _prep_graph(edge_index)
    params = _prep_params(inputs)
    core_inputs = _prep_core_inputs(meta, x, edge_attr, params)

    NT = meta["NT"]
    if NT not in _CACHE:
        nc = build_kernel(NT)
        nc.compile()
        _CACHE[NT] = nc
    nc = _CACHE[NT]

    res = run_bass_kernel_spmd(nc, core_inputs, core_ids=list(range(NCORES)))
    y = np.concatenate([res.results[r]["y"][0] for r in range(NCORES)])
    return y[:N].reshape(N, 1).astype(np.float32)


if __name__ == "__main__":
    import reference
    ins = {k: np.asarray(v) for k, v in reference.setup_inputs().items()}
    out = kernel(**ins)
    print(out.shape, out.dtype, out[:4, 0])


# revision 17
# speedup vs baseline: 1.3612x; 1.0908x over previous
"""GATNet (3-layer GAT with edge features) on 8 Trainium2 NeuronCores.

Strategy (dst-sharded, edge-sorted, host-assisted):
  - Nodes padded to N_PAD=20480 -> 160 chunks of 128 nodes; 20 chunks/core.
  - Edges + one self-loop per node, sorted by dst; every edge lands on the
    core owning its dst chunk => all segment softmax/aggregation core-local.
  - Host precomputes (not on the device critical path):
      * a_e = edge_attr @ (We folded with att_e) for all 3 layers, incl. the
        self-loop rows (PyG fill_value='mean' -> per-dst mean of real a_e)
        and -1e4 on padding slots (kills exp); staged as bf16 input.
      * one-hot scatter matrices M1 (edge->dst, fp8, SBUF-resident) and M1T
        (dst->edge, fp8, streamed per chunk) for the PE-matmul scatter/
        broadcast; identity I128 fp8.
      * layer-1 projections: hs1 table [h|a_s] (bf16, replicated) and a_d1.
  - Per layer: (L2/L3 only) cores project their own 2560-node shard with one
    fused matmul (att_s/att_d folded into W on host), AllGather the bf16
    [h|a_s] table; then per chunk:
      dma_gather h-rows by src; alpha = M1T@a_d + I@a_e + I@a_s accumulated
      on the PE into PSUM; DVE leaky-relu; Act engine exp with C-broadcast
      (L3 split Act/Pool); DVE 2x multiply exh = ex*h in place; PE matmuls
      M1^T @ [exh|ex] scatter-add numerator+denominator into PSUM; DVE
      divide+relu.  Softmax max-subtraction dropped (alpha is O(1)).
"""

import math
import sys

import numpy as np

sys.path.insert(0, "/opt/trn_rl_repo")

import ml_dtypes  # noqa: E402

import concourse.bacc as bacc  # noqa: E402
import concourse.bass as bass  # noqa: E402
import concourse.mybir as mybir  # noqa: E402
import concourse.tile as tile  # noqa: E402
from concourse.bass_utils import run_bass_kernel_spmd  # noqa: E402

bf16 = ml_dtypes.bfloat16
fp8 = ml_dtypes.float8_e3m4

N = 20000
E = 320000
FIN = 16
ED = 22
NEG = 0.2
NCORES = 8
NPAD = 20480
NCH = NPAD // 128 // NCORES     # 20 chunks per core
SHARD = NPAD // NCORES          # 2560 own nodes per core
LAYERS = [(16, 8, 32), (256, 8, 32), (256, 12, 64)]
ROWW = [384, 384, 896]          # hs row: [h(HC) | a_s(H) | pad], bytes%256==0
AEW = [8, 8, 12]
AE_TOT = 28
F32 = mybir.dt.float32
BF16 = mybir.dt.bfloat16
FP8 = mybir.dt.float8e3
I16 = mybir.dt.int16
AF = mybir.ActivationFunctionType
ALU = mybir.AluOpType
L3_ACT_FRAC = 0.62              # fraction of L3 ex-broadcast done on Act


# ============================ host-side prep ============================

def _relabel(deg):
    """LPT-pack nodes into 128-node chunks balancing in-degree sums, so the
    max per-chunk edge count (and thus NT) is minimized.  Returns old2new."""
    import heapq
    nchunks = NPAD // 128
    order = np.argsort(-deg, kind="stable")
    heap = [(0, c) for c in range(nchunks)]
    heapq.heapify(heap)
    counts = np.zeros(nchunks, np.int64)
    old2new = np.zeros(NPAD, np.int64)
    for n in order:
        load, c = heapq.heappop(heap)
        old2new[n] = c * 128 + counts[c]
        counts[c] += 1
        if counts[c] < 128:
            heapq.heappush(heap, (load + int(deg[n]), c))
    return old2new


def _prep_graph(edge_index):
    src0 = edge_index[0].astype(np.int64)
    dst0 = edge_index[1].astype(np.int64)
    deg = np.bincount(dst0, minlength=NPAD)
    old2new = _relabel(deg)
    src = old2new[src0]
    dst = old2new[dst0]
    order = np.argsort(dst, kind="stable")
    src_s = src[order]
    dst_s = dst[order]
    cnt = np.bincount(dst, minlength=NPAD)

    nchunks = NPAD // 128
    chunk_of = dst_s // 128
    chunk_cnt = np.bincount(chunk_of, minlength=nchunks) + 128
    NT = int(math.ceil(chunk_cnt.max() / 128.0))
    SL = NT * 128

    tot = nchunks * SL
    g_src = np.zeros(tot, np.int64)
    g_dstloc = np.zeros(tot, np.int64)
    g_isself = np.zeros(tot, np.bool_)
    g_eaidx = np.full(tot, -1, np.int64)

    starts = np.searchsorted(chunk_of, np.arange(nchunks))
    ends = np.searchsorted(chunk_of, np.arange(nchunks) + 1)
    for c in range(nchunks):
        base = c * SL
        r0, r1 = int(starts[c]), int(ends[c])
        d_loc = dst_s[r0:r1] - c * 128
        nreal = r1 - r0
        seg_cnt = np.bincount(d_loc, minlength=128)
        blk_off = np.zeros(128, np.int64)
        np.cumsum(seg_cnt[:-1] + 1, out=blk_off[1:])
        within = np.arange(nreal) - np.repeat(np.cumsum(seg_cnt) - seg_cnt, seg_cnt)
        pos = base + blk_off[d_loc] + within
        g_src[pos] = src_s[r0:r1]
        g_dstloc[pos] = d_loc
        g_eaidx[pos] = order[r0:r1]
        pos_self = base + blk_off + seg_cnt
        g_src[pos_self] = c * 128 + np.arange(128)
        g_dstloc[pos_self] = np.arange(128)
        g_isself[pos_self] = True

    g_ispad = (g_eaidx < 0) & ~g_isself
    return {"NT": NT, "SL": SL, "cnt": cnt, "src": g_src,
            "dstloc": g_dstloc, "isself": g_isself, "eaidx": g_eaidx,
            "ispad": g_ispad, "old2new": old2new}


def _wrap_idx(idx):
    n = idx.shape[0]
    w = idx.astype(np.int16).reshape(n // 16, 16).T
    return np.tile(w, (8, 1))


def _prep_params(kw):
    p = {}
    wered = []
    for li, (fin, H, C) in enumerate(LAYERS):
        i = li + 1
        W = kw[f"W{i}"].astype(np.float32)
        We = kw[f"We{i}"].astype(np.float32)
        ats = kw[f"as{i}"].astype(np.float32)
        atd = kw[f"ad{i}"].astype(np.float32)
        ate = kw[f"ae{i}"].astype(np.float32)
        Was = np.einsum("dhc,hc->dh", W.reshape(fin, H, C), ats)
        Wad = np.einsum("dhc,hc->dh", W.reshape(fin, H, C), atd)
        if li > 0:
            p[f"Wall{i}"] = np.concatenate([W, Was, Wad], axis=1).astype(bf16)
            bias = np.zeros((1, H * C + H), np.float32)
            bias[0, :H * C] = kw[f"b{i}"].astype(np.float32)
            p[f"bias{i}"] = bias.astype(bf16)
        else:
            # layer 1 projections are host-computed from x directly
            p["bias1"] = np.concatenate(
                [kw["b1"].astype(np.float32), np.zeros(H, np.float32)]
            ).reshape(1, -1).astype(bf16)
            p["_W1"] = W
            p["_Was1"] = Was
            p["_Wad1"] = Wad
        wered.append(np.einsum("dhc,hc->dh", We.reshape(ED, H, C), ate))
    p["_wered"] = np.concatenate(wered, axis=1).astype(np.float32)  # [ED,28]
    Wf = kw["Wf"].astype(np.float32).reshape(-1)
    p["wf1"] = np.ascontiguousarray(Wf[0:256].reshape(2, 128).T).astype(bf16)
    p["wf2"] = np.ascontiguousarray(Wf[256:512].reshape(2, 128).T).astype(bf16)
    p["wf3"] = np.ascontiguousarray(Wf[512:1280].reshape(6, 128).T).astype(bf16)
    p["bf"] = kw["bf"].astype(np.float32).reshape(1, 1)
    p["ident"] = np.eye(128, dtype=fp8)
    return p


def _prep_core_inputs(meta, x, edge_attr, params):
    NT, SL = meta["NT"], meta["SL"]
    npc = NCH * SL

    # ---- layer-1 host projections (node-relabeled order) ----
    x_pad = np.zeros((NPAD, FIN), np.float32)
    x_pad[meta["old2new"][:N]] = x
    h1 = x_pad @ params["_W1"]                        # [NPAD, 256]
    as1 = x_pad @ params["_Was1"]                     # [NPAD, 8]
    ad1 = x_pad @ params["_Wad1"]                     # [NPAD, 8]
    hs1 = np.zeros((NPAD, ROWW[0]), np.float32)
    hs1[:, 0:256] = h1
    hs1[:, 256:264] = as1
    hs1_b = hs1.astype(bf16)

    # ---- a_e for all edge slots (3 layers fused), incl. self means/pad ----
    ae_edge = edge_attr.astype(np.float32) @ params["_wered"]   # [E, 28]
    cnt = np.maximum(meta["cnt"][:, None], 1.0)
    sums = np.zeros((NPAD, AE_TOT), np.float32)
    dst_full = np.zeros(0)
    # mean per dst over real incoming edges
    # (use eaidx/dstloc info: real slots have eaidx>=0)
    real = meta["eaidx"] >= 0
    slot_chunk = np.arange(160 * SL) // SL
    dst_node = slot_chunk * 128 + meta["dstloc"]
    np.add.at(sums, dst_node[real], ae_edge[meta["eaidx"][real]])
    mean_ae = sums / cnt
    ae_slot = np.zeros((160 * SL, AE_TOT), np.float32)
    ae_slot[real] = ae_edge[meta["eaidx"][real]]
    ae_slot[meta["isself"]] = mean_ae[dst_node[meta["isself"]]]
    ae_slot[meta["ispad"]] = -1e4
    # layer-1 a_d[dst] is host-known: fold it into layer-1 a_e
    ae_slot[:, 0:8] += ad1[dst_node]

    # ---- one-hot scatter matrices ----
    dl = meta["dstloc"].reshape(160, NT, 128)         # [chunk, t, e]
    eye = np.eye(128, dtype=fp8)
    m1_all = eye[dl]                                  # [chunk, t, e(part), n]
    ins = []
    for r in range(NCORES):
        sl = slice(r * npc, (r + 1) * npc)
        chs = slice(r * NCH, (r + 1) * NCH)
        idx16 = _wrap_idx(meta["src"][sl])
        # M1: [128(e), NCH*NT*128(n)]
        m1 = np.ascontiguousarray(
            m1_all[chs].transpose(2, 0, 1, 3).reshape(128, NCH * NT * 128))
        # M1T: [128(n), NCH*NT*128(e)]
        m1t = np.ascontiguousarray(
            m1_all[chs].transpose(3, 0, 1, 2).reshape(128, NCH * NT * 128))
        d = {"idx16": idx16, "m1": m1, "m1t": m1t, "hs1": hs1_b}
        off = 0
        for li, w in enumerate(AEW):
            ae_c = np.ascontiguousarray(
                ae_slot[sl, off:off + w].reshape(NCH * NT, 128, w)
                .transpose(1, 0, 2).reshape(128, NCH * NT * w)).astype(bf16)
            d[f"ae{li + 1}c"] = ae_c
            off += w
        d.update({k: v for k, v in params.items() if not k.startswith("_")})
        ins.append(d)
    return ins


# ============================ device kernel ============================

def build_kernel(NT, nch=NCH, use_cc=True):
    NCHl = nch
    SHARDl = NCHl * 128
    NPADl = SHARDl * NCORES
    SL = NT * 128
    npc = NCHl * SL
    TPC = NCHl * NT

    nc = bacc.Bacc("TRN2", num_devices=NCORES)

    d_idx = nc.dram_tensor("idx16", [128, npc // 16], I16, kind="ExternalInput")
    d_m1 = nc.dram_tensor("m1", [128, npc], FP8, kind="ExternalInput")
    d_m1t = nc.dram_tensor("m1t", [128, npc], FP8, kind="ExternalInput")
    d_aec = [nc.dram_tensor(f"ae{li + 1}c", [128, TPC * AEW[li]], BF16,
                            kind="ExternalInput") for li in range(3)]
    d_hs1 = nc.dram_tensor("hs1", [NPADl, ROWW[0]], BF16, kind="ExternalInput")
    d_ident = nc.dram_tensor("ident", [128, 128], FP8, kind="ExternalInput")
    d_Wall, d_bias = {}, {}
    for li, (fin, H, C) in enumerate(LAYERS):
        if li > 0:
            d_Wall[li] = nc.dram_tensor(f"Wall{li + 1}", [fin, H * C + 2 * H],
                                        BF16, kind="ExternalInput")
        d_bias[li] = nc.dram_tensor(f"bias{li + 1}", [1, H * C + H], BF16,
                                    kind="ExternalInput")
    d_wf = [nc.dram_tensor(f"wf{i + 1}", [128, nb], BF16, kind="ExternalInput")
            for i, nb in enumerate((2, 2, 6))]
    d_bf = nc.dram_tensor("bf", [1, 1], F32, kind="ExternalInput")
    d_y = nc.dram_tensor("y", [1, SHARDl], F32, kind="ExternalOutput")

    with tile.TileContext(nc) as tc:
        with tc.tile_pool(name="const", bufs=1) as cpool, \
             tc.tile_pool(name="lay", bufs=1) as lpool, \
             tc.tile_pool(name="work", bufs=2) as wpool, \
             tc.tile_pool(name="gbuf", bufs=3) as gpool, \
             tc.tile_pool(name="exbuf", bufs=2) as xpool, \
             tc.tile_pool(name="psbig", bufs=2, space="PSUM") as psb, \
             tc.tile_pool(name="pssm", bufs=2, space="PSUM") as pss:

            # internal DRAM
            d_hs_in = [None] + [nc.dram_tensor(f"d_hs_in{li}", [SHARDl, ROWW[li]],
                                               BF16) for li in (1, 2)]
            d_hs = [None] + [nc.dram_tensor(f"d_hs{li}", [NPADl, ROWW[li]], BF16)
                             for li in (1, 2)]
            d_x = [nc.dram_tensor(f"d_x{li}",
                                  [SHARDl, LAYERS[li][1] * LAYERS[li][2]], BF16)
                   for li in range(3)]

            # ---------- constants ----------
            t_ones = cpool.tile([1, 128], BF16)
            nc.vector.memset(t_ones[:], 1.0)
            t_ident = cpool.tile([128, 128], FP8)
            nc.sync.dma_start(out=t_ident[:], in_=d_ident[:])
            t_idx = cpool.tile([128, npc // 16], I16)
            nc.sync.dma_start(out=t_idx[:], in_=d_idx[:])

            # ---------- layers ----------
            for li, (fin, H, C) in enumerate(LAYERS):
                HC = H * C
                RW_ = ROWW[li]
                AEw = AE_TOT
                NDW = HC + H
                PJW = HC + 2 * H
                nkb = max(fin // 128, 1)
                KP = min(fin, 128)

                t_bias = lpool.tile([1, NDW], BF16, tag=f"bias{li}")
                nc.sync.dma_start(out=t_bias[:], in_=d_bias[li][:])
                t_ad = lpool.tile([128, NCHl * H], BF16, tag=f"ad{li}")

                # ---- phase A: own-shard projections -> hs shard + a_d ----
                if li == 0:
                    hs_tab = d_hs1
                else:
                    t_W = lpool.tile([KP, nkb, PJW], BF16, tag=f"W{li}")
                    nc.sync.dma_start(
                        out=t_W[:],
                        in_=d_Wall[li][:].rearrange("(b p) w -> p b w", p=KP))
                    t_xT = lpool.tile([128, nkb * SHARDl], BF16, tag=f"xT{li}")
                    for b in range(nkb):
                        nc.sync.dma_start(
                            out=t_xT[:, b * SHARDl:(b + 1) * SHARDl],
                            in_=d_x[li - 1][:, b * 128:(b + 1) * 128],
                            transpose=True)
                    for ch in range(NCHl):
                        p_h = psb.tile([128, 1024], F32, space="PSUM", tag="big")
                        for b in range(nkb):
                            xsl = t_xT[:KP, b * SHARDl + ch * 128:
                                       b * SHARDl + ch * 128 + 128]
                            for c0 in range(0, PJW, 512):
                                c1 = min(c0 + 512, PJW)
                                nc.tensor.matmul(
                                    out=p_h[:, c0:c1],
                                    lhsT=xsl,
                                    rhs=t_W[:, b, c0:c1],
                                    start=(b == 0), stop=(b == nkb - 1))
                        t_hs = wpool.tile([128, RW_], BF16, tag="hsrow")
                        nc.scalar.copy(out=t_hs[:, 0:HC + H], in_=p_h[:, 0:HC + H])
                        nc.sync.dma_start(out=d_hs_in[li][ch * 128:(ch + 1) * 128, :],
                                          in_=t_hs[:])
                        nc.vector.tensor_copy(
                            t_ad[:, ch * H:(ch + 1) * H],
                            p_h[:, HC + H:HC + 2 * H])

                    if use_cc:
                        nc.gpsimd.collective_compute(
                            "AllGather", ALU.bypass,
                            replica_groups=[list(range(NCORES))],
                            ins=[d_hs_in[li].ap().opt()],
                            outs=[d_hs[li].ap().opt()])
                    else:
                        nc.gpsimd.dma_start(out=d_hs[li][0:SHARDl, :],
                                            in_=d_hs_in[li][:])
                    hs_tab = d_hs[li]

                # ---- phase B: edges ----
                for ch in range(NCHl):
                    t_g = gpool.tile([128, NT, RW_], BF16, tag="G")
                    nc.gpsimd.dma_gather(t_g[:], hs_tab[:],
                                         t_idx[:, ch * SL // 16:(ch + 1) * SL // 16],
                                         SL, SL, RW_, single_packet=False)
                    AEw_l = AEW[li]
                    t_ae = wpool.tile([128, NT, AEw_l], BF16, tag="aeL")
                    nc.sync.dma_start(
                        out=t_ae[:],
                        in_=d_aec[li][:, ch * NT * AEw_l:(ch + 1) * NT * AEw_l]
                        .rearrange("p (t h) -> p t h", t=NT))
                    if li > 0:
                        t_m1t = wpool.tile([128, SL], FP8, tag="m1t")
                        nc.sync.dma_start(out=t_m1t[:],
                                          in_=d_m1t[:, ch * SL:(ch + 1) * SL])
                    t_m1 = wpool.tile([128, SL], FP8, tag="m1c")
                    nc.sync.dma_start(out=t_m1[:],
                                      in_=d_m1[:, ch * SL:(ch + 1) * SL])

                    # three passes over half-chunks so each engine's in-order
                    # queue never parks behind a cross-engine wait:
                    #   pass 1 (PE): alpha = a_d[dst] + a_e + a_s[src]
                    #   pass 2 (DVE/Act/Pool): lrelu -> exp + C-replicate
                    #   pass 3 (DVE then PE): exh multiply -> scatter matmuls
                    NH = (NT + 1) // 2
                    halves = [(0, NH), (NH, NT)]
                    p_als, t_lrs, t_exCs = [], [], []
                    for h0, h1 in halves:
                        nh = h1 - h0
                        p_al = pss.tile([128, NH, H], F32, space="PSUM",
                                        tag="psmA")
                        p_als.append(p_al)
                        for t in range(h0, h1):
                            tt = t - h0
                            if li > 0:
                                nc.tensor.matmul(
                                    out=p_al[:, tt, :],
                                    lhsT=t_m1t[:, t * 128:(t + 1) * 128],
                                    rhs=t_ad[:, ch * H:(ch + 1) * H],
                                    start=True, stop=False)
                            nc.tensor.matmul(out=p_al[:, tt, :],
                                             lhsT=t_ident[:],
                                             rhs=t_ae[:, t, :],
                                             start=(li == 0), stop=False)
                            nc.tensor.matmul(out=p_al[:, tt, :],
                                             lhsT=t_ident[:],
                                             rhs=t_g[:, t, HC:HC + H],
                                             start=False, stop=True)
                    for hi, (h0, h1) in enumerate(halves):
                        nh = h1 - h0
                        p_al = p_als[hi]
                        t_al = wpool.tile([128, NH, H], F32, tag="al")
                        nc.vector.tensor_copy(t_al[:, 0:nh], p_al[:, 0:nh])
                        t_lr = wpool.tile([128, NH, H], F32, tag="lr")
                        nc.vector.scalar_tensor_tensor(
                            out=t_lr[:, 0:nh], in0=t_al[:, 0:nh], scalar=NEG,
                            in1=t_al[:, 0:nh], op0=ALU.mult, op1=ALU.max)
                        t_exC = xpool.tile([128, NH, H, C], BF16, tag="exC")
                        t_exCs.append(t_exC)
                        if li < 2:
                            nc.scalar.activation(
                                t_exC[:, 0:nh],
                                t_lr[:, 0:nh].unsqueeze(-1)
                                .broadcast_to([128, nh, H, C]),
                                AF.Exp)
                        else:
                            t_ex = wpool.tile([128, NH, H], BF16, tag="ex")
                            nc.scalar.activation(t_ex[:, 0:nh], t_lr[:, 0:nh],
                                                 AF.Exp)
                            ta = max(1, int(nh * L3_ACT_FRAC))
                            nc.scalar.copy(
                                out=t_exC[:, 0:ta],
                                in_=t_ex[:, 0:ta].unsqueeze(-1)
                                .broadcast_to([128, ta, H, C]))
                            nc.gpsimd.tensor_copy(
                                t_exC[:, ta:nh],
                                t_ex[:, ta:nh].unsqueeze(-1)
                                .broadcast_to([128, nh - ta, H, C]))

                    p_nd = psb.tile([128, 1024], F32, space="PSUM", tag="big")
                    for c0 in range(0, NDW, 512):
                        c1 = min(c0 + 512, NDW)
                        nc.tensor.matmul(out=p_nd[:, c0:c1], lhsT=t_ones[:],
                                         rhs=t_bias[:, c0:c1], start=True,
                                         stop=False)
                    for hi, (h0, h1) in enumerate(halves):
                        nh = h1 - h0
                        t_exC = t_exCs[hi]
                        # exh = ex*h in place (2x mode: all bf16 packed)
                        nc.vector.tensor_tensor(
                            out=t_g[:, h0:h1, 0:HC],
                            in0=t_g[:, h0:h1, 0:HC],
                            in1=t_exC[:, 0:nh].rearrange("p t h c -> p t (h c)"),
                            op=ALU.mult)
                        # denominator cols: ex (pick c=0 stride-C view)
                        nc.vector.tensor_copy(t_g[:, h0:h1, HC:NDW],
                                              t_exC[:, 0:nh, :, 0])
                        # scatter this half: NUMDEN += sum_t M1_t.T @ exh_t
                        for t in range(h0, h1):
                            for c0 in range(0, NDW, 512):
                                c1 = min(c0 + 512, NDW)
                                nc.tensor.matmul(
                                    out=p_nd[:, c0:c1],
                                    lhsT=t_m1[:, t * 128:(t + 1) * 128],
                                    rhs=t_g[:, t, c0:c1],
                                    start=False, stop=(t == NT - 1))
                    # x = relu(num/den)
                    t_rec = wpool.tile([128, H], F32, tag="rec")
                    nc.vector.reciprocal(t_rec[:], p_nd[:, HC:NDW])
                    t_x = wpool.tile([128, HC], BF16, tag="xout")
                    nc.vector.scalar_tensor_tensor(
                        out=t_x[:].rearrange("p (h c) -> p h c", h=H),
                        in0=p_nd[:, 0:HC].rearrange("p (h c) -> p h c", h=H),
                        scalar=0.0, op0=ALU.max, op1=ALU.mult,
                        in1=t_rec[:].unsqueeze(-1).broadcast_to([128, H, C]))
                    nc.sync.dma_start(out=d_x[li][ch * 128:(ch + 1) * 128, :],
                                      in_=t_x[:])

            # ---------- final: y = sigmoid(concat(x1,x2,x3) @ Wf + bf) ----------
            t_wf = [lpool.tile([128, nb], BF16, tag=f"wf{i}", name=f"t_wf{i}")
                    for i, nb in enumerate((2, 2, 6))]
            for i in range(3):
                nc.sync.dma_start(out=t_wf[i][:], in_=d_wf[i][:])
            t_bf = lpool.tile([1, 1], F32, tag="bf")
            nc.sync.dma_start(out=t_bf[:], in_=d_bf[:])
            for g in range(SHARDl // 512):
                p_y = pss.tile([1, 512], F32, space="PSUM", tag="psmB")
                first = True
                for li in range(3):
                    nbl = (LAYERS[li][1] * LAYERS[li][2]) // 128
                    for b in range(nbl):
                        t_xg = wpool.tile([128, 512], BF16, tag="xg")
                        nc.sync.dma_start(
                            out=t_xg[:],
                            in_=d_x[li][g * 512:(g + 1) * 512,
                                        b * 128:(b + 1) * 128],
                            transpose=True)
                        nc.tensor.matmul(out=p_y[:], lhsT=t_wf[li][:, b:b + 1],
                                         rhs=t_xg[:], start=first,
                                         stop=(li == 2 and b == nbl - 1))
                        first = False
                t_y = wpool.tile([1, 512], F32, tag="yrow")
                nc.scalar.activation(t_y[:], p_y[:], AF.Sigmoid, bias=t_bf[:])
                nc.sync.dma_start(out=d_y[0:1, g * 512:(g + 1) * 512], in_=t_y[:])

    return nc


# ============================ public entry ============================

_CACHE = {}


def kernel(**inputs):
    x = np.asarray(inputs["x"], np.float32)
    edge_index = np.asarray(inputs["edge_index"])
    edge_attr = np.asarray(inputs["edge_attr"], np.float32)

    meta = _prep_graph(edge_index)
    params = _prep_params(inputs)
    core_inputs = _prep_core_inputs(meta, x, edge_attr, params)

    NT = meta["NT"]
    if NT not in _CACHE:
        nc = build_kernel(NT)
        nc.compile()
        _CACHE[NT] = nc
    nc = _CACHE[NT]

    res = run_bass_kernel_spmd(nc, core_inputs, core_ids=list(range(NCORES)))
    y = np.concatenate([res.results[r]["y"][0] for r in range(NCORES)])
    return y[meta["old2new"][:N]].reshape(N, 1).astype(np.float32)


if __name__ == "__main__":
    import reference
    ins = {k: np.asarray(v) for k, v in reference.setup_inputs().items()}
    out = kernel(**ins)
    print(out.shape, out.dtype, out[:4, 0])
